# revision 1
# baseline (speedup 1.0000x reference)
"""Trainium2 Bass kernel for nn_BrainGeneratorModel (bias-field corrupt + per-sample
separable Gaussian blur + label LUT remap), 8-core data/spatial parallel.

Sharding: 8 cores = (sample b in 0..3) x (H-half in 0..1). Each core processes a
[D=192, H=96(+12 halo), W=192] subvolume of one sample plus its label slice.

Per-core pipeline (all blurs as PE matmuls against host-built banded matrices):
  A) stream d-batches: bias matmul (K=4) -> exp (ACT) -> x*expb (GPSIMD, bf16 out)
     -> H-blur matmul -> PE transpose (w onto partitions) -> W-blur matmul
     -> Y2 scratch in DRAM (f32)
  B) stream h-batches from Y2: PE transpose (d onto partitions) -> D-blur matmul
     -> img out (PSUM->DRAM direct)
  L) labels: 32-entry LUT as 32 fused is_equal*T[k] tensor_scalar + accumulate
     scalar_tensor_tensor passes on DVE (int16).
"""

import sys

for _p in ("/opt/trn_rl_repo",):
    if _p not in sys.path:
        sys.path.insert(0, _p)

import numpy as np
import ml_dtypes

import concourse.bass as bass
import concourse.mybir as mybir
import concourse.bacc as bacc
import concourse.tile as tile
from concourse.bass_utils import run_bass_kernel_spmd

F32 = mybir.dt.float32
BF16 = mybir.dt.bfloat16
I16 = mybir.dt.int16
I32 = mybir.dt.int32

B, C, D, H, W = 4, 1, 192, 192, 192
SMALL = 4
BIAS_STD = 0.7
MAX_SIGMA = 3.0
TRUNCATE = 4.0
K = 2 * int(TRUNCATE * MAX_SIGMA) + 1  # 25
P = K // 2  # 12
N_LABELS = 32
TABLE = 128

HC = 96            # interior H rows per core
HS = 120           # slab rows = HC + 2*P
DB = 8             # d-batch size (stage A)
NB_A = D // DB     # 24 batches
HB = 8             # h-batch size (stage B)
NB_B = HC // HB    # 12 batches
FA = DB * W        # 1536 stage-A free size
FLAB = D * HC * W // 128  # 27648 label cols per partition

_CACHE = {}


def _lin_weights(n_in, n_out):
    pos = np.linspace(0.0, n_in - 1.0, n_out, dtype=np.float64)
    i0 = np.clip(np.floor(pos).astype(np.int64), 0, n_in - 2)
    f = pos - i0
    Wm = np.zeros((n_out, n_in), np.float64)
    r = np.arange(n_out)
    np.add.at(Wm, (r, i0), 1.0 - f)
    np.add.at(Wm, (r, i0 + 1), f)
    return Wm


def _gauss_kernels(sigma3):
    """sigma3 [3] -> [3, K] kernels exactly as reference."""
    ar = np.arange(K, dtype=np.float64) - K // 2
    out = np.zeros((3, K), np.float64)
    for i, sg in enumerate(sigma3):
        s = max(float(sg), 1e-3)
        g = np.exp(-0.5 * ar * ar / (s * s))
        g = g / g.sum()
        if float(sg) >= 0.01:
            out[i] = g
        else:
            out[i, K // 2] = 1.0
    return out


def _edge_folded_toeplitz(g, n):
    """[n, n] matrix M with out[j] = sum_i M[i, j] * x[i], replicate padding."""
    M = np.zeros((n, n), np.float64)
    for j in range(n):
        for t in range(K):
            src = min(max(j + t - P, 0), n - 1)
            M[src, j] += g[t]
    return M


def _slab_toeplitz(g):
    """[HS, HC]: slab rows (pre-clipped by host) -> interior outputs."""
    M = np.zeros((HS, HC), np.float64)
    for j in range(HC):
        for t in range(K):
            M[j + t, j] += g[t]
    return M


def _build_program():
    nc = bacc.Bacc("TRN2", target_bir_lowering=False, debug=False)

    # ---- external inputs (per core) ----
    xs_h = nc.dram_tensor("xs", [D, HS, W], F32, kind="ExternalInput")
    c_h = nc.dram_tensor("cydw", [4, D * W], BF16, kind="ExternalInput")
    wht_h = nc.dram_tensor("wht", [4, HS], BF16, kind="ExternalInput")
    gh_h = nc.dram_tensor("gh", [HS, HC], BF16, kind="ExternalInput")
    gw_h = nc.dram_tensor("gw", [W, W], BF16, kind="ExternalInput")
    gd_h = nc.dram_tensor("gd", [D, D], BF16, kind="ExternalInput")
    lab_h = nc.dram_tensor("lab", [128, FLAB], I16, kind="ExternalInput")
    tab_h = nc.dram_tensor("tab", [128, N_LABELS], F32, kind="ExternalInput")
    id_h = nc.dram_tensor("idm", [128, 128], BF16, kind="ExternalInput")
    idf_h = nc.dram_tensor("idmf", [128, 128], F32, kind="ExternalInput")

    # ---- external outputs ----
    img_h = nc.dram_tensor("img", [D, HC, W], F32, kind="ExternalOutput")
    labo_h = nc.dram_tensor("labo", [128, FLAB], I16, kind="ExternalOutput")

    with tile.TileContext(nc) as tc:
        with (
            tc.tile_pool(name="consts", bufs=1) as cst,
            tc.tile_pool(name="sxp", bufs=2) as sxp,
            tc.tile_pool(name="cbp", bufs=2) as cbp,
            tc.tile_pool(name="ebp", bufs=2) as ebp,
            tc.tile_pool(name="xbp", bufs=2) as xbp,
            tc.tile_pool(name="xhp", bufs=2) as xhp,
            tc.tile_pool(name="zwp", bufs=2) as zwp,
            tc.tile_pool(name="ybp", bufs=2) as ybp,
            tc.tile_pool(name="zdp", bufs=2) as zdp,
            tc.tile_pool(name="zvp", bufs=2) as zvp,
            tc.tile_pool(name="zip", bufs=2) as zip_,
            tc.tile_pool(name="labp", bufs=1) as labp,
            tc.tile_pool(name="ps", bufs=8, space="PSUM") as psp,
            tc.tile_pool(name="dram", bufs=1, space="DRAM") as drp,
        ):
            # ---- constants to SBUF ----
            ght = cst.tile([HS, HC], BF16)
            nc.sync.dma_start(ght[:], gh_h.ap())
            gwa = cst.tile([128, W], BF16)
            nc.sync.dma_start(gwa[:], gw_h.ap()[0:128, :])
            gwb = cst.tile([64, W], BF16)
            nc.sync.dma_start(gwb[:], gw_h.ap()[128:192, :])
            gda = cst.tile([128, D], BF16)
            nc.sync.dma_start(gda[:], gd_h.ap()[0:128, :])
            gdb = cst.tile([64, D], BF16)
            nc.sync.dma_start(gdb[:], gd_h.ap()[128:192, :])
            whtt = cst.tile([4, HS], BF16)
            nc.sync.dma_start(whtt[:], wht_h.ap())
            idt = cst.tile([128, 128], BF16)
            nc.sync.dma_start(idt[:], id_h.ap())
            idft = cst.tile([128, 128], F32)
            nc.sync.dma_start(idft[:], idf_h.ap())
            tabt = cst.tile([128, N_LABELS], F32)
            nc.sync.dma_start(tabt[:], tab_h.ap())

            y2 = drp.tile([W, HC * D], F32)  # scratch [w', h', d]

            # ================= labels (DVE only, 2 chunks) =================
            FL2 = FLAB // 2
            for cc in range(2):
                lsl = slice(cc * FL2, (cc + 1) * FL2)
                lt = labp.tile([128, FL2], I16, tag="lt")
                nc.sync.dma_start(lt[:], lab_h.ap()[:, lsl])
                acc = labp.tile([128, FL2], I16, tag="acc")
                ek = labp.tile([128, FL2], I16, tag="ek")
                nc.vector.tensor_scalar(
                    acc[:], lt[:], 0, tabt[:, 0:1],
                    mybir.AluOpType.is_equal, mybir.AluOpType.mult)
                for k in range(1, N_LABELS):
                    nc.vector.tensor_scalar(
                        ek[:], lt[:], k, tabt[:, k:k + 1],
                        mybir.AluOpType.is_equal, mybir.AluOpType.mult)
                    nc.vector.scalar_tensor_tensor(
                        acc[:], ek[:], 1, acc[:],
                        mybir.AluOpType.mult, mybir.AluOpType.add)
                nc.sync.dma_start(labo_h.ap()[:, lsl], acc[:])

            # ================= stage A =================
            for ib in range(NB_A):
                d0 = ib * DB
                sx = sxp.tile([HS, FA], F32)
                # src: partition h (stride W), free (d: stride HS*W, w: 1)
                nc.sync.dma_start(
                    sx[:],
                    bass.AP(xs_h, d0 * HS * W, [[W, HS], [HS * W, DB], [1, W]]),
                )
                cb = cbp.tile([4, FA], BF16)
                nc.sync.dma_start(cb[:], c_h.ap()[:, d0 * W:(d0 + DB) * W])

                xb = xbp.tile([HS, FA], BF16)
                for q in range(FA // 512):
                    sl = slice(q * 512, (q + 1) * 512)
                    psb = psp.tile([HS, 512], F32, tag="ps")
                    nc.tensor.matmul(psb[:], whtt[:], cb[:, sl], start=True, stop=True)
                    eb = ebp.tile([HS, 512], F32)
                    nc.scalar.activation(eb[:], psb[:], mybir.ActivationFunctionType.Exp)
                    nc.gpsimd.tensor_tensor(xb[:, sl], sx[:, sl], eb[:], mybir.AluOpType.mult)

                xh = xhp.tile([HC, FA], BF16)
                for q in range(FA // 512):
                    sl = slice(q * 512, (q + 1) * 512)
                    psh = psp.tile([HC, 512], F32, tag="ps")
                    nc.tensor.matmul(psh[:], ght[:], xb[:, sl], start=True, stop=True)
                    nc.scalar.copy(xh[:, sl], psh[:])

                # T1: w onto partitions. zw free layout: (h', dl): idx = hp*DB + dl
                zwa = zwp.tile([128, HC * DB], BF16, tag="zwa")
                zwb = zwp.tile([64, HC * DB], BF16, tag="zwb")
                for g in range(DB // 4):
                    pta = psp.tile([128, 4 * HC], BF16, tag="ps")
                    ptb = psp.tile([64, 4 * HC], BF16, tag="ps")
                    for t in range(4):
                        dl = g * 4 + t
                        nc.tensor.transpose(
                            pta[:, t * HC:(t + 1) * HC],
                            xh[:, dl * W: dl * W + 128], idt[0:HC, 0:HC])
                        nc.tensor.transpose(
                            ptb[:, t * HC:(t + 1) * HC],
                            xh[:, dl * W + 128: dl * W + 192], idt[0:HC, 0:HC])
                    # copy psum->zw with (t outer, h' inner) -> (dl, h'*DB+dl)
                    nc.scalar.copy(
                        zwa[:].rearrange("p (h d) -> p d h", d=DB)[:, g * 4:(g + 1) * 4, :],
                        pta[:].rearrange("p (t h) -> p t h", t=4),
                    )
                    nc.scalar.copy(
                        zwb[:].rearrange("p (h d) -> p d h", d=DB)[:, g * 4:(g + 1) * 4, :],
                        ptb[:].rearrange("p (t h) -> p t h", t=4),
                    )

                # W-blur -> SBUF staging -> y2 [w', h', d]
                nfree = HC * DB  # 768
                for m in range(2):
                    msl = slice(m * 96, (m + 1) * 96)
                    zv = zvp.tile([96, nfree], F32, tag="zv")
                    for q in range(nfree // 384):
                        sl = slice(q * 384, (q + 1) * 384)  # 48 h' x DB dl
                        psw = psp.tile([96, 384], F32, tag="ps")
                        nc.tensor.matmul(psw[:], gwa[:, msl], zwa[:, sl], start=True, stop=False)
                        nc.tensor.matmul(psw[:], gwb[:, msl], zwb[:, sl], start=False, stop=True)
                        nc.scalar.copy(zv[:, sl], psw[:])
                    # zv free = (h' 96, dl 8); y2 free = h'*D + d
                    nc.sync.dma_start(
                        bass.AP(y2.tensor,
                                y2[:].offset + m * 96 * HC * D + d0,
                                [[HC * D, 96], [D, HC], [1, DB]]),
                        zv[:],
                    )

            # ================= stage B =================
            for jb in range(NB_B):
                h0 = jb * HB
                yba = ybp.tile([96, HB * D], F32, tag="yba")
                ybb = ybp.tile([96, HB * D], F32, tag="ybb")
                nc.sync.dma_start(yba[:], bass.AP(y2.tensor, y2[:].offset + h0 * D,
                                                  [[HC * D, 96], [1, HB * D]]))
                nc.sync.dma_start(ybb[:], bass.AP(y2.tensor,
                                                  y2[:].offset + 96 * HC * D + h0 * D,
                                                  [[HC * D, 96], [1, HB * D]]))
                zda = zdp.tile([128, HB * W], BF16, tag="zda")
                zdb = zdp.tile([64, HB * W], BF16, tag="zdb")
                for g in range(HB // 2):
                    pta = psp.tile([128, 384], F32, tag="ps")
                    ptb = psp.tile([64, 384], F32, tag="ps")
                    for t in range(2):
                        hl = g * 2 + t
                        nc.tensor.transpose(
                            pta[:, t * 192 + 0: t * 192 + 96],
                            yba[:, hl * D + 0: hl * D + 128], idft[0:96, 0:96])
                        nc.tensor.transpose(
                            pta[:, t * 192 + 96: t * 192 + 192],
                            ybb[:, hl * D + 0: hl * D + 128], idft[0:96, 0:96])
                        nc.tensor.transpose(
                            ptb[:, t * 192 + 0: t * 192 + 96],
                            yba[:, hl * D + 128: hl * D + 192], idft[0:96, 0:96])
                        nc.tensor.transpose(
                            ptb[:, t * 192 + 96: t * 192 + 192],
                            ybb[:, hl * D + 128: hl * D + 192], idft[0:96, 0:96])
                    nc.scalar.copy(zda[:, g * 384:(g + 1) * 384], pta[:])
                    nc.scalar.copy(zdb[:, g * 384:(g + 1) * 384], ptb[:])

                # D-blur, img out [d', (hl, w)] via SBUF staging
                for m in range(2):
                    msl = slice(m * 96, (m + 1) * 96)
                    zi = zip_.tile([96, HB * W], F32, tag="zi")
                    for q in range(HB * W // 512):
                        sl = slice(q * 512, (q + 1) * 512)
                        psd = psp.tile([96, 512], F32, tag="ps")
                        nc.tensor.matmul(psd[:], gda[:, msl], zda[:, sl], start=True, stop=False)
                        nc.tensor.matmul(psd[:], gdb[:, msl], zdb[:, sl], start=False, stop=True)
                        nc.scalar.copy(zi[:, sl], psd[:])
                    nc.sync.dma_start(
                        bass.AP(img_h, m * 96 * HC * W + h0 * W,
                                [[HC * W, 96], [1, HB * W]]),
                        zi[:],
                    )
    nc.compile()
    return nc


def _host_prep(x, small_bias, sigma01, labels, source_values, dest_values):
    Wd = _lin_weights(SMALL, D)
    Whm = _lin_weights(SMALL, H)
    Wwm = _lin_weights(SMALL, W)
    eye_bf = np.eye(128, dtype=ml_dtypes.bfloat16)
    eye_f32 = np.eye(128, dtype=np.float32)

    mapping = np.zeros(TABLE, np.int32)
    mapping[np.asarray(source_values, np.int64)] = np.asarray(dest_values, np.int64).astype(np.int32)
    tabf = mapping[:N_LABELS].astype(np.float32)
    tab_rep = np.broadcast_to(tabf, (128, N_LABELS)).copy()

    in_maps = []
    for c in range(8):
        b, half = c // 2, c % 2
        h0 = half * HC
        hidx = np.clip(np.arange(h0 - P, h0 + HC + P), 0, H - 1)

        xs = np.ascontiguousarray(np.asarray(x[b, 0], np.float32)[:, hidx, :])

        sm = np.asarray(small_bias[b, 0], np.float64) * BIAS_STD
        Cydw = np.einsum("xyz,dx,wz->ydw", sm, Wd, Wwm).reshape(4, D * W)
        WhT = np.ascontiguousarray(Whm[hidx, :].T)

        g3 = _gauss_kernels(np.asarray(sigma01[b], np.float64) * MAX_SIGMA)
        Gh = _slab_toeplitz(g3[1])
        Gw = _edge_folded_toeplitz(g3[2], W)
        Gd = _edge_folded_toeplitz(g3[0], D)

        lab = np.asarray(labels[b, 0][:, h0:h0 + HC, :], np.int16).reshape(128, FLAB)

        in_maps.append({
            "xs": xs,
            "cydw": Cydw.astype(ml_dtypes.bfloat16),
            "wht": WhT.astype(ml_dtypes.bfloat16),
            "gh": Gh.astype(ml_dtypes.bfloat16),
            "gw": Gw.astype(ml_dtypes.bfloat16),
            "gd": Gd.astype(ml_dtypes.bfloat16),
            "lab": np.ascontiguousarray(lab),
            "tab": tab_rep,
            "idm": eye_bf,
            "idmf": eye_f32,
        })
    return in_maps


def kernel(x, small_bias, sigma01, labels, source_values, dest_values):
    if "nc" not in _CACHE:
        _CACHE["nc"] = _build_program()
    nc = _CACHE["nc"]

    in_maps = _host_prep(x, small_bias, sigma01, labels, source_values, dest_values)
    res = run_bass_kernel_spmd(nc, in_maps, core_ids=list(range(8)))

    img = np.empty((B, C, D, H, W), np.float32)
    labels_out = np.empty((B, C, D, H, W), np.int32)
    for c in range(8):
        b, half = c // 2, c % 2
        h0 = half * HC
        r = res.results[c]
        img[b, 0, :, h0:h0 + HC, :] = r["img"].reshape(D, HC, W)
        labels_out[b, 0, :, h0:h0 + HC, :] = (
            r["labo"].reshape(D, HC, W).astype(np.int32))
    return img, labels_out



# revision 59
# speedup vs baseline: 2.8912x; 2.8912x over previous
"""Trainium2 Bass kernel for nn_BrainGeneratorModel (bias-field corrupt + per-sample
separable Gaussian blur + label LUT remap), 8-core data/spatial parallel.

Sharding: 8 cores = (sample b in 0..3) x (H-half in 0..1). Each core processes a
[D=192, H=96(+12 halo), W=192] subvolume of one sample plus its label slice.

Per-core pipeline:
  A) stream d-batches: bias matmul (K=4) -> exp (ACT, bf16) -> x*expb (DVE)
     -> H-blur matmul (banded 120->96) -> PE transposes (w onto partitions,
     two 108-row windows) -> banded W-blur (one 108-contraction matmul per
     96-row output tile) -> y2 kept resident in SBUF as bf16 [w', (h', d)]
  B) stream h-batches from SBUF y2: PE transposes (d onto partitions, two
     108-row windows) -> banded D-blur -> img DMA'd f32 directly from PSUM.
  L) labels: 16-entry packed-int16 LUT (C16[h] = T[2h] | T[2h+1]<<7), split
     three ways: DVE-direct compare chain, PE-hybrid (DVE compares + int16
     identity-matmul PSUM accumulation), and GPSIMD ap_gather.
"""

import sys

for _p in ("/opt/trn_rl_repo",):
    if _p not in sys.path:
        sys.path.insert(0, _p)

import numpy as np
import ml_dtypes

import concourse.bass as bass
import concourse.mybir as mybir
import concourse.bacc as bacc
import concourse.tile as tile
from concourse.bass_utils import run_bass_kernel_spmd

F32 = mybir.dt.float32
BF16 = mybir.dt.bfloat16
I16 = mybir.dt.int16
I32 = mybir.dt.int32
A = mybir.AluOpType

B, C, D, H, W = 4, 1, 192, 192, 192
SMALL = 4
BIAS_STD = 0.7
MAX_SIGMA = 3.0
TRUNCATE = 4.0
K = 2 * int(TRUNCATE * MAX_SIGMA) + 1  # 25
P = K // 2  # 12
N_LABELS = 32
TABLE = 128

HC = 96            # interior H rows per core
HS = 120           # slab rows = HC + 2*P
DB = 8             # d-batch size (stage A)
NB_A = D // DB     # 24 batches
HB = 8             # h-batch size (stage B)
NB_B = HC // HB    # 12 batches
FA = DB * W        # 1536 stage-A free size
WIN = 108          # banded blur input window (96 + 12)
FLAB = D * HC * W // 128  # 27648 label cols per partition

# --- label split across engines (cols) ---
FL_DVE = 8448      # DVE-direct share (16-entry packed LUT)
FL_PE = 8192       # PE share (32 scaled-one-hot matmul accumulation)
FL_G = FLAB - FL_DVE - FL_PE  # 10752 -> gpsimd ap_gather share
LCH = 1920         # chunk cols for the DVE label path
PCH = 1024         # chunk cols for the PE label path
GCH = 128          # cols per ap_gather instruction (out free = 16*GCH)
GLD = 1024         # cols per gather-path input DMA

_CACHE = {}

import os as _os
_NO_LABELS = bool(int(_os.environ.get("KERN_NO_LABELS", "0")))
_NO_IMG = bool(int(_os.environ.get("KERN_NO_IMG", "0")))


def _lin_weights(n_in, n_out):
    pos = np.linspace(0.0, n_in - 1.0, n_out, dtype=np.float64)
    i0 = np.clip(np.floor(pos).astype(np.int64), 0, n_in - 2)
    f = pos - i0
    Wm = np.zeros((n_out, n_in), np.float64)
    r = np.arange(n_out)
    np.add.at(Wm, (r, i0), 1.0 - f)
    np.add.at(Wm, (r, i0 + 1), f)
    return Wm


def _gauss_kernels(sigma3):
    ar = np.arange(K, dtype=np.float64) - K // 2
    out = np.zeros((3, K), np.float64)
    for i, sg in enumerate(sigma3):
        s = max(float(sg), 1e-3)
        g = np.exp(-0.5 * ar * ar / (s * s))
        g = g / g.sum()
        if float(sg) >= 0.01:
            out[i] = g
        else:
            out[i, K // 2] = 1.0
    return out


def _edge_folded_toeplitz(g, n):
    """[n, n] matrix M with out[j] = sum_i M[i, j] * x[i], replicate padding."""
    M = np.zeros((n, n), np.float64)
    for j in range(n):
        for t in range(K):
            src = min(max(j + t - P, 0), n - 1)
            M[src, j] += g[t]
    return M


def _slab_toeplitz(g):
    """[HS, HC]: slab rows (pre-clipped by host) -> interior outputs."""
    M = np.zeros((HS, HC), np.float64)
    for j in range(HC):
        for t in range(K):
            M[j + t, j] += g[t]
    return M


def _build_program():
    nc = bacc.Bacc("TRN2", target_bir_lowering=False, debug=False)

    # ---- external inputs (per core) ----
    xs_h = nc.dram_tensor("xs", [HS, D * W], BF16, kind="ExternalInput")
    c_h = nc.dram_tensor("cydw", [4, D * W], BF16, kind="ExternalInput")
    wht_h = nc.dram_tensor("wht", [4, HS], BF16, kind="ExternalInput")
    gh_h = nc.dram_tensor("gh", [HS, HC], BF16, kind="ExternalInput")
    gw0_h = nc.dram_tensor("gw0", [WIN, HC], BF16, kind="ExternalInput")
    gw1_h = nc.dram_tensor("gw1", [WIN, HC], BF16, kind="ExternalInput")
    gd0_h = nc.dram_tensor("gd0", [WIN, HC], BF16, kind="ExternalInput")
    gd1_h = nc.dram_tensor("gd1", [WIN, HC], BF16, kind="ExternalInput")
    lab_h = nc.dram_tensor("lab", [128, FLAB], I16, kind="ExternalInput")
    c16_h = nc.dram_tensor("c16", [128, 16], F32, kind="ExternalInput")
    tabf_h = nc.dram_tensor("tabf", [128, N_LABELS], F32, kind="ExternalInput")
    tab32_h = nc.dram_tensor("tab32", [128, N_LABELS], I32, kind="ExternalInput")
    idbf_h = nc.dram_tensor("idbf", [128, 128], BF16, kind="ExternalInput")

    # ---- external outputs ----
    img_h = nc.dram_tensor("img", [D, HC, W], BF16, kind="ExternalOutput")
    labo_h = nc.dram_tensor("labo", [128, FL_DVE], I16, kind="ExternalOutput")
    labp_h = nc.dram_tensor("labp", [128, FL_PE], BF16, kind="ExternalOutput")
    labg_h = nc.dram_tensor("labg", [8, 16 * FL_G], I32, kind="ExternalOutput")

    from contextlib import ExitStack
    with tile.TileContext(nc) as tc:
        with ExitStack() as stack:
            cst = stack.enter_context(tc.tile_pool(name="consts", bufs=1))
            sxp = stack.enter_context(tc.tile_pool(name="sxp", bufs=2))
            cbp = stack.enter_context(tc.tile_pool(name="cbp", bufs=2))
            ebp = stack.enter_context(tc.tile_pool(name="ebp", bufs=6))
            xbp = stack.enter_context(tc.tile_pool(name="xbp", bufs=2))
            xhp = stack.enter_context(tc.tile_pool(name="xhp", bufs=2))
            zwp = stack.enter_context(tc.tile_pool(name="zwp", bufs=2))
            y2p = stack.enter_context(tc.tile_pool(name="y2p", bufs=1))
            zdp = stack.enter_context(tc.tile_pool(name="zdp", bufs=3))
            zip_ = stack.enter_context(tc.tile_pool(name="zip", bufs=4))
            lp = stack.enter_context(tc.tile_pool(name="lp", bufs=2))
            ltmp = stack.enter_context(tc.tile_pool(name="ltmp", bufs=1))
            dkp = stack.enter_context(tc.tile_pool(name="dkp", bufs=6))
            glp = stack.enter_context(tc.tile_pool(name="glp", bufs=2))
            gop = stack.enter_context(tc.tile_pool(name="gop", bufs=3))
            psp = stack.enter_context(tc.tile_pool(name="ps", bufs=4, space="PSUM"))
            pslp = stack.enter_context(tc.tile_pool(name="psl", bufs=2, space="PSUM"))
            psdp = stack.enter_context(tc.tile_pool(name="psd", bufs=2, space="PSUM"))
            # ---- constants to SBUF ----
            ght = cst.tile([HS, HC], BF16)
            nc.sync.dma_start(ght[:], gh_h.ap())
            gw0 = cst.tile([WIN, HC], BF16)
            nc.sync.dma_start(gw0[:], gw0_h.ap())
            gw1 = cst.tile([WIN, HC], BF16)
            nc.sync.dma_start(gw1[:], gw1_h.ap())
            gd0 = cst.tile([WIN, HC], BF16)
            nc.sync.dma_start(gd0[:], gd0_h.ap())
            gd1 = cst.tile([WIN, HC], BF16)
            nc.sync.dma_start(gd1[:], gd1_h.ap())
            whtt = cst.tile([4, HS], BF16)
            nc.sync.dma_start(whtt[:], wht_h.ap())
            c16t = cst.tile([128, 16], F32)
            nc.sync.dma_start(c16t[:], c16_h.ap())
            tabf = cst.tile([128, N_LABELS], F32)
            nc.sync.dma_start(tabf[:], tabf_h.ap())
            tab32 = cst.tile([128, N_LABELS], I32)
            nc.sync.dma_start(tab32[:], tab32_h.ap())
            idbf = cst.tile([128, 128], BF16)
            nc.sync.dma_start(idbf[:], idbf_h.ap())

            # y2 SBUF-resident: w' halves on partitions 0..95, free (h', d)
            y2a = y2p.tile([HC, HC * D], BF16, tag="y2a")
            y2b = y2p.tile([HC, HC * D], BF16, tag="y2b")

            def emit_label_dve(s0, fc):
                """DVE-direct 16-entry packed LUT on labo cols [s0, s0+fc)."""
                lt = lp.tile([128, fc], I16, tag="lt")
                nc.scalar.dma_start(lt[:], lab_h.ap()[:, s0:s0 + fc])
                hh = ltmp.tile([128, fc], I16, tag="hh")
                sh = ltmp.tile([128, fc], I16, tag="sh")
                acc = ltmp.tile([128, fc], I16, tag="acc")
                ek = ltmp.tile([128, fc], I16, tag="ek")
                o16 = lp.tile([128, fc], I16, tag="o16")
                nc.vector.tensor_scalar(hh[:], lt[:], 1, None, A.logical_shift_right)
                nc.vector.tensor_scalar(sh[:], lt[:], 1, None, A.bitwise_and)
                nc.vector.tensor_scalar(sh[:], sh[:], 7, None, A.mult)
                nc.vector.tensor_scalar(acc[:], hh[:], 0, c16t[:, 0:1], A.is_equal, A.mult)
                for k in range(1, 16):
                    nc.vector.tensor_scalar(ek[:], hh[:], k, c16t[:, k:k + 1], A.is_equal, A.mult)
                    # disjoint-one-hot accumulate: paired 32-bit bitwise OR
                    # (2 int16 lanes per op; int32 adds would round via fp32)
                    nc.vector.tensor_tensor(acc[:].bitcast(I32), acc[:].bitcast(I32),
                                            ek[:].bitcast(I32), A.bitwise_or)
                nc.vector.tensor_tensor(acc[:], acc[:], sh[:], A.logical_shift_right)
                nc.vector.tensor_scalar(o16[:], acc[:], 127, None, A.bitwise_and)
                nc.sync.dma_start(labo_h.ap()[:, s0:s0 + fc], o16[:])

            def emit_label_pe(s0, fc):
                """PE path: 32 one-hot compares (DVE, table value folded in,
                <=99 so bf16-exact) -> identity-matmul PSUM accumulation.
                Compares are emitted LOOKAHEAD passes ahead of their matmuls
                so the in-order PE queue doesn't stall on DVE."""
                LOOKAHEAD = 4
                lt = lp.tile([128, fc], I16, tag="lt")
                nc.scalar.dma_start(lt[:], lab_h.ap()[:, s0:s0 + fc])
                ob = lp.tile([128, fc], BF16, tag="ob")
                nq = (fc + 511) // 512
                psls = []
                for _ in range(nq):
                    pslt = pslp.tile([128, 512], F32, tag="psl")
                    psls.append(pslt)
                dks = {}
                def emit_compare(k):
                    dk = dkp.tile([128, fc], BF16, tag="dk")
                    nc.vector.tensor_scalar(dk[:], lt[:], k, tabf[:, k:k + 1],
                                            A.is_equal, A.mult)
                    dks[k] = dk
                for k in range(LOOKAHEAD):
                    emit_compare(k)
                for k in range(N_LABELS):
                    if k + LOOKAHEAD < N_LABELS:
                        emit_compare(k + LOOKAHEAD)
                    dk = dks.pop(k)
                    for q in range(nq):
                        qn = min(512, fc - q * 512)
                        nc.tensor.matmul(psls[q][:, :qn], idbf[:],
                                         dk[:, q * 512:q * 512 + qn],
                                         start=(k == 0), stop=(k == N_LABELS - 1))
                for q in range(nq):
                    qn = min(512, fc - q * 512)
                    nc.vector.tensor_copy(ob[:, q * 512:q * 512 + qn], psls[q][:, :qn])
                nc.sync.dma_start(labp_h.ap()[:, s0 - FL_DVE:s0 - FL_DVE + fc], ob[:])

            def emit_label_gather(s0, fc):
                """GPSIMD ap_gather over gather-share cols [s0, s0+fc)."""
                ltg = glp.tile([128, fc], I16, tag="ltg")
                base = FL_DVE + FL_PE
                nc.sync.dma_start(ltg[:], lab_h.ap()[:, base + s0:base + s0 + fc])
                pend_g = None
                for g0 in list(range(0, fc, GCH)) + [None]:
                    if g0 is not None:
                        og = gop.tile([128, 16 * GCH], I32, tag="og")
                        nc.gpsimd.ap_gather(og[:], tab32[:], ltg[:, g0:g0 + GCH],
                                            channels=128, num_elems=N_LABELS, d=1,
                                            num_idxs=16 * GCH)
                    if pend_g is not None:
                        og_p, g0_p = pend_g
                        pstep = og_p[:].ap[0][0]
                        nc.sync.dma_start(
                            labg_h.ap()[:, 16 * (s0 + g0_p):16 * (s0 + g0_p + GCH)],
                            bass.AP(og_p.tensor, og_p[:].offset,
                                    [[pstep * 16, 8], [1, 16 * GCH]]),
                        )
                    pend_g = (og, g0) if g0 is not None else None

            ldve_chunks = [(s, min(LCH, FL_DVE - s)) for s in range(0, FL_DVE, LCH)]
            lpe_chunks = [(FL_DVE + s, min(PCH, FL_PE - s)) for s in range(0, FL_PE, PCH)]
            lg_chunks = [(s, min(GLD, FL_G - s)) for s in range(0, FL_G, GLD)]
            li = [0, 0, 0]

            def drip_labels(fd, fp, fg):
                # per-path completion targets (fractions of each chunk list)
                if _NO_LABELS:
                    return
                while li[0] < len(ldve_chunks) * fd:
                    emit_label_dve(*ldve_chunks[li[0]])
                    li[0] += 1
                while li[1] < len(lpe_chunks) * fp:
                    emit_label_pe(*lpe_chunks[li[1]])
                    li[1] += 1
                while li[2] < len(lg_chunks) * fg:
                    emit_label_gather(*lg_chunks[li[2]])
                    li[2] += 1

            # ================= stage A (software-pipelined) =================
            def phase_a1(ib):
                """load + bias matmul + exp; returns (sx, ebs)."""
                d0 = ib * DB
                sx = sxp.tile([HS, FA], BF16)
                nc.sync.dma_start(sx[:], xs_h.ap()[:, d0 * W:(d0 + DB) * W])
                cb = cbp.tile([4, FA], BF16)
                nc.sync.dma_start(cb[:], c_h.ap()[:, d0 * W:(d0 + DB) * W])
                ebs = []
                for q in range(FA // 512):
                    sl = slice(q * 512, (q + 1) * 512)
                    psb = psp.tile([HS, 512], F32, tag="ps")
                    nc.tensor.matmul(psb[:], whtt[:], cb[:, sl], start=True, stop=True)
                    eb = ebp.tile([HS, 512], BF16, tag="eb")
                    nc.scalar.activation(eb[:], psb[:], mybir.ActivationFunctionType.Exp)
                    ebs.append(eb)
                return sx, ebs

            def phase_a2(ib, sx, ebs):
                """mult, H-blur, T1, W-blur -> y2."""
                d0 = ib * DB
                xb = xbp.tile([HS, FA], BF16)
                for q in range(FA // 512):
                    sl = slice(q * 512, (q + 1) * 512)
                    nc.vector.tensor_tensor(xb[:, sl], sx[:, sl], ebs[q][:], A.mult)

                xh = xhp.tile([HC, FA], BF16)
                for q in range(FA // 512):
                    sl = slice(q * 512, (q + 1) * 512)
                    psh = psp.tile([HC, 512], F32, tag="ps")
                    nc.tensor.matmul(psh[:], ght[:], xb[:, sl], start=True, stop=True)
                    nc.scalar.copy(xh[:, sl], psh[:])

                # T1: w onto partitions, two 108-row windows; zw free = (dl, h')
                zw0 = zwp.tile([WIN, DB * HC], BF16, tag="zw0")
                zw1 = zwp.tile([WIN, DB * HC], BF16, tag="zw1")
                for half in range(2):
                    pt0 = psp.tile([WIN, 4 * HC], BF16, tag="ps")
                    pt1 = psp.tile([WIN, 4 * HC], BF16, tag="ps")
                    for t in range(4):
                        dl = half * 4 + t
                        nc.tensor.transpose(
                            pt0[:, t * HC:(t + 1) * HC],
                            xh[:, dl * W: dl * W + WIN], idbf[0:HC, 0:HC])
                        nc.tensor.transpose(
                            pt1[:, t * HC:(t + 1) * HC],
                            xh[:, dl * W + 84: dl * W + 192], idbf[0:HC, 0:HC])
                    nc.vector.tensor_copy(zw0[:, half * 4 * HC:(half + 1) * 4 * HC], pt0[:])
                    nc.vector.tensor_copy(zw1[:, half * 4 * HC:(half + 1) * 4 * HC], pt1[:])

                # banded W-blur -> y2 (SBUF), out tile m covers w' in [96m, 96m+96)
                for m, (gwm, zwm, y2m) in enumerate(((gw0, zw0, y2a), (gw1, zw1, y2b))):
                    for q in range(2):
                        sl = slice(q * 4 * HC, (q + 1) * 4 * HC)
                        psw = psp.tile([HC, 4 * HC], F32, tag="ps")
                        nc.tensor.matmul(psw[:], gwm[:], zwm[:, sl], start=True, stop=True)
                        # psw free = (dl 4, h' 96) -> y2 free (h'*D + d) at d0+q*4
                        nc.scalar.copy(
                            bass.AP(y2m.tensor,
                                    y2m[:].offset + d0 + q * 4,
                                    [[y2m[:].ap[0][0], HC], [1, 4], [D, HC]]),
                            psw[:],
                        )

            pend = None
            for ib in range(NB_A + 1 if not _NO_IMG else 0):
                cur = phase_a1(ib) if ib < NB_A else None
                if pend is not None:
                    phase_a2(ib - 1, *pend)
                    u = ib / NB_A
                    drip_labels(u * 0.26, u, u * 0.67)
                pend = cur

            # ================= stage B (software-pipelined) =================
            def phase_b1(jb):
                """T2 transposes + zd copies; returns (zd0, zd1)."""
                h0 = jb * HB
                zd0 = zdp.tile([WIN, HB * W], BF16, tag="zd0")
                zd1 = zdp.tile([WIN, HB * W], BF16, tag="zd1")
                for half in range(2):
                    pt0 = psp.tile([WIN, 4 * 2 * HC], BF16, tag="ps")
                    pt1 = psp.tile([WIN, 4 * 2 * HC], BF16, tag="ps")
                    for t in range(4):
                        hl = half * 4 + t
                        hbase = (h0 + hl) * D
                        nc.tensor.transpose(
                            pt0[:, t * 2 * HC: t * 2 * HC + HC],
                            y2a[:, hbase: hbase + WIN], idbf[0:HC, 0:HC])
                        nc.tensor.transpose(
                            pt0[:, t * 2 * HC + HC: (t + 1) * 2 * HC],
                            y2b[:, hbase: hbase + WIN], idbf[0:HC, 0:HC])
                        nc.tensor.transpose(
                            pt1[:, t * 2 * HC: t * 2 * HC + HC],
                            y2a[:, hbase + 84: hbase + 192], idbf[0:HC, 0:HC])
                        nc.tensor.transpose(
                            pt1[:, t * 2 * HC + HC: (t + 1) * 2 * HC],
                            y2b[:, hbase + 84: hbase + 192], idbf[0:HC, 0:HC])
                    nc.scalar.copy(
                        zd0[:, half * 4 * W:(half + 1) * 4 * W], pt0[:])
                    nc.scalar.copy(
                        zd1[:, half * 4 * W:(half + 1) * 4 * W], pt1[:])
                return zd0, zd1

            def phase_b2(jb, zd0, zd1):
                """banded D-blur + img out; out tile m = d' in [96m, 96m+96)."""
                h0 = jb * HB
                for m, (gdm, zdm) in enumerate(((gd0, zd0), (gd1, zd1))):
                    for q in range(3):
                        sl = slice(q * 512, (q + 1) * 512)
                        psd = psdp.tile([HC, 512], F32, tag="psd")
                        nc.tensor.matmul(psd[:], gdm[:], zdm[:, sl], start=True, stop=True)
                        zi = zip_.tile([HC, 512], BF16, tag="zi")
                        nc.scalar.copy(zi[:], psd[:])
                        nc.sync.dma_start(
                            bass.AP(img_h, m * HC * HC * W + h0 * W + q * 512,
                                    [[HC * W, HC], [1, 512]]),
                            zi[:],
                        )

            pendb = None
            for jb in range(NB_B + 1 if not _NO_IMG else 0):
                curb = phase_b1(jb) if jb < NB_B else None
                if pendb is not None:
                    phase_b2(jb - 1, *pendb)
                    v = jb / NB_B
                    drip_labels(0.26 + v * 0.74, 1.0, 0.67 + v * 0.33)
                pendb = curb

            drip_labels(1.0, 1.0, 1.0)

    nc.compile()
    return nc


def _host_prep(x, small_bias, sigma01, labels, source_values, dest_values):
    Wd = _lin_weights(SMALL, D)
    Whm = _lin_weights(SMALL, H)
    Wwm = _lin_weights(SMALL, W)
    eyebf = np.eye(128, dtype=ml_dtypes.bfloat16)

    mapping = np.zeros(TABLE, np.int64)
    mapping[np.asarray(source_values, np.int64)] = np.asarray(dest_values, np.int64)
    T = mapping[:N_LABELS]
    C16 = (T[0::2] | (T[1::2] << 7)).astype(np.float32)
    c16_rep = np.broadcast_to(C16, (128, 16)).copy()
    tabf_rep = np.broadcast_to(T.astype(np.float32), (128, N_LABELS)).copy()
    tab32_rep = np.broadcast_to(T.astype(np.int32), (128, N_LABELS)).copy()

    in_maps = []
    for c in range(8):
        b, half = c // 2, c % 2
        h0 = half * HC
        hidx = np.clip(np.arange(h0 - P, h0 + HC + P), 0, H - 1)

        # x slab, h-major layout [HS, D, W] in bf16
        xs = np.ascontiguousarray(
            np.asarray(x[b, 0], np.float32)[:, hidx, :].transpose(1, 0, 2)
        ).astype(ml_dtypes.bfloat16).reshape(HS, D * W)

        sm = np.asarray(small_bias[b, 0], np.float64) * BIAS_STD
        Cydw = np.einsum("xyz,dx,wz->ydw", sm, Wd, Wwm).reshape(4, D * W)
        WhT = np.ascontiguousarray(Whm[hidx, :].T)

        g3 = _gauss_kernels(np.asarray(sigma01[b], np.float64) * MAX_SIGMA)
        Gh = _slab_toeplitz(g3[1])
        Mw = _edge_folded_toeplitz(g3[2], W)
        Md = _edge_folded_toeplitz(g3[0], D)
        Gw0 = Mw[0:WIN, 0:HC]
        Gw1 = Mw[84:192, HC:192]
        Gd0 = Md[0:WIN, 0:HC]
        Gd1 = Md[84:192, HC:192]

        lab = np.asarray(labels[b, 0][:, h0:h0 + HC, :], np.int16).reshape(128, FLAB)

        in_maps.append({
            "xs": xs,
            "cydw": Cydw.astype(ml_dtypes.bfloat16),
            "wht": WhT.astype(ml_dtypes.bfloat16),
            "gh": Gh.astype(ml_dtypes.bfloat16),
            "gw0": Gw0.astype(ml_dtypes.bfloat16),
            "gw1": Gw1.astype(ml_dtypes.bfloat16),
            "gd0": Gd0.astype(ml_dtypes.bfloat16),
            "gd1": Gd1.astype(ml_dtypes.bfloat16),
            "lab": np.ascontiguousarray(lab),
            "c16": c16_rep,
            "tabf": tabf_rep,
            "tab32": tab32_rep,
            "idbf": eyebf,
        })
    return in_maps


def kernel(x, small_bias, sigma01, labels, source_values, dest_values):
    if "nc" not in _CACHE:
        _CACHE["nc"] = _build_program()
    nc = _CACHE["nc"]

    in_maps = _host_prep(x, small_bias, sigma01, labels, source_values, dest_values)
    res = run_bass_kernel_spmd(nc, in_maps, core_ids=list(range(8)))

    img = np.empty((B, C, D, H, W), np.float32)
    labels_out = np.empty((B, C, D, H, W), np.int32)
    for c in range(8):
        b, half = c // 2, c % 2
        h0 = half * HC
        r = res.results[c]
        img[b, 0, :, h0:h0 + HC, :] = r["img"].reshape(D, HC, W).astype(np.float32)
        lo = np.empty((128, FLAB), np.int32)
        lo[:, :FL_DVE] = r["labo"].astype(np.int32)
        lo[:, FL_DVE:FL_DVE + FL_PE] = np.asarray(r["labp"], np.float32).astype(np.int32)
        # gather share: labg [8, 16*FL_G], row g holds group g's elements in
        # (slot-major, partition-interleaved) order
        lg = r["labg"].astype(np.int32).reshape(8, FL_G, 16)
        for g in range(8):
            lo[16 * g:16 * g + 16, FL_DVE + FL_PE:] = lg[g].T
        labels_out[b, 0, :, h0:h0 + HC, :] = lo.reshape(D, HC, W)
    return img, labels_out


# revision 69
# speedup vs baseline: 3.0856x; 1.0672x over previous
"""Trainium2 Bass kernel for nn_BrainGeneratorModel (bias-field corrupt + per-sample
separable Gaussian blur + label LUT remap), 8-core data/spatial parallel.

Sharding: 8 cores = (sample b in 0..3) x (H-half in 0..1). Each core processes a
[D=192, H=96(+12 halo), W=192] subvolume of one sample plus its label slice.

Per-core pipeline:
  A) stream d-batches: bias matmul (K=4) -> exp (ACT, bf16) -> x*expb (DVE)
     -> H-blur matmul (banded 120->96) -> PE transposes (w onto partitions,
     two 108-row windows) -> banded W-blur (one 108-contraction matmul per
     96-row output tile) -> y2 kept resident in SBUF as bf16 [w', (h', d)]
  B) stream h-batches from SBUF y2: PE transposes (d onto partitions, two
     108-row windows) -> banded D-blur -> img out as bf16.
  L) labels split three ways: DVE-direct 16-entry packed-int16 LUT
     (C16[h] = T[2h] | T[2h+1]<<7, one-hot compare chain with paired 32-bit
     OR accumulate), PE path (32 one-hot compares with the bf16-exact table
     value folded in, accumulated via identity matmuls in PSUM), and GPSIMD
     ap_gather (per-16-partition-group shared-index gather, host deinterleaves).
"""

import sys

for _p in ("/opt/trn_rl_repo",):
    if _p not in sys.path:
        sys.path.insert(0, _p)

import numpy as np
import ml_dtypes

import concourse.bass as bass
import concourse.mybir as mybir
import concourse.bacc as bacc
import concourse.tile as tile
from concourse.bass_utils import run_bass_kernel_spmd

F32 = mybir.dt.float32
BF16 = mybir.dt.bfloat16
I16 = mybir.dt.int16
I32 = mybir.dt.int32
A = mybir.AluOpType

B, C, D, H, W = 4, 1, 192, 192, 192
SMALL = 4
BIAS_STD = 0.7
MAX_SIGMA = 3.0
TRUNCATE = 4.0
K = 2 * int(TRUNCATE * MAX_SIGMA) + 1  # 25
P = K // 2  # 12
N_LABELS = 32
TABLE = 128

HC = 96            # interior H rows per core
HS = 120           # slab rows = HC + 2*P
DB = 8             # d-batch size (stage A)
NB_A = D // DB     # 24 batches
HB = 8             # h-batch size (stage B)
NB_B = HC // HB    # 12 batches
FA = DB * W        # 1536 stage-A free size
WIN = 108          # banded blur input window (96 + 12)
FLAB = D * HC * W // 128  # 27648 label cols per partition

# --- label split across engines (cols) ---
FL_DVE = 7680      # DVE-direct share (16-entry packed LUT)
FL_PE = 8960       # PE share (32 scaled-one-hot matmul accumulation)
FL_G = FLAB - FL_DVE - FL_PE  # 10752 -> gpsimd ap_gather share
LCH = 1920         # chunk cols for the DVE label path
PCH = 1024         # chunk cols for the PE label path
GCH = 128          # cols per ap_gather instruction (out free = 16*GCH)
GLD = 1024         # cols per gather-path input DMA

_CACHE = {}

import os as _os
_NO_LABELS = bool(int(_os.environ.get("KERN_NO_LABELS", "0")))
_NO_IMG = bool(int(_os.environ.get("KERN_NO_IMG", "0")))


def _lin_weights(n_in, n_out):
    pos = np.linspace(0.0, n_in - 1.0, n_out, dtype=np.float64)
    i0 = np.clip(np.floor(pos).astype(np.int64), 0, n_in - 2)
    f = pos - i0
    Wm = np.zeros((n_out, n_in), np.float64)
    r = np.arange(n_out)
    np.add.at(Wm, (r, i0), 1.0 - f)
    np.add.at(Wm, (r, i0 + 1), f)
    return Wm


def _gauss_kernels(sigma3):
    ar = np.arange(K, dtype=np.float64) - K // 2
    out = np.zeros((3, K), np.float64)
    for i, sg in enumerate(sigma3):
        s = max(float(sg), 1e-3)
        g = np.exp(-0.5 * ar * ar / (s * s))
        g = g / g.sum()
        if float(sg) >= 0.01:
            out[i] = g
        else:
            out[i, K // 2] = 1.0
    return out


def _edge_folded_toeplitz(g, n):
    """[n, n] matrix M with out[j] = sum_i M[i, j] * x[i], replicate padding."""
    M = np.zeros((n, n), np.float64)
    for j in range(n):
        for t in range(K):
            src = min(max(j + t - P, 0), n - 1)
            M[src, j] += g[t]
    return M


def _slab_toeplitz(g):
    """[HS, HC]: slab rows (pre-clipped by host) -> interior outputs."""
    M = np.zeros((HS, HC), np.float64)
    for j in range(HC):
        for t in range(K):
            M[j + t, j] += g[t]
    return M


def _build_program():
    nc = bacc.Bacc("TRN2", target_bir_lowering=False, debug=False)

    # ---- external inputs (per core) ----
    xs_h = nc.dram_tensor("xs", [HS, D * W], BF16, kind="ExternalInput")
    c_h = nc.dram_tensor("cydw", [4, D * W], BF16, kind="ExternalInput")
    wht_h = nc.dram_tensor("wht", [4, HS], BF16, kind="ExternalInput")
    gh_h = nc.dram_tensor("gh", [HS, HC], BF16, kind="ExternalInput")
    gw0_h = nc.dram_tensor("gw0", [WIN, HC], BF16, kind="ExternalInput")
    gw1_h = nc.dram_tensor("gw1", [WIN, HC], BF16, kind="ExternalInput")
    gd0_h = nc.dram_tensor("gd0", [WIN, HC], BF16, kind="ExternalInput")
    gd1_h = nc.dram_tensor("gd1", [WIN, HC], BF16, kind="ExternalInput")
    lab_h = nc.dram_tensor("lab", [128, FLAB], I16, kind="ExternalInput")
    c16_h = nc.dram_tensor("c16", [128, 16], F32, kind="ExternalInput")
    tabf_h = nc.dram_tensor("tabf", [128, N_LABELS], F32, kind="ExternalInput")
    tab32_h = nc.dram_tensor("tab32", [128, N_LABELS], I32, kind="ExternalInput")
    idbf_h = nc.dram_tensor("idbf", [128, 128], BF16, kind="ExternalInput")

    # ---- external outputs ----
    img_h = nc.dram_tensor("img", [D, HC, W], BF16, kind="ExternalOutput")
    labo_h = nc.dram_tensor("labo", [128, FL_DVE], I16, kind="ExternalOutput")
    labp_h = nc.dram_tensor("labp", [128, FL_PE], BF16, kind="ExternalOutput")
    labg_h = nc.dram_tensor("labg", [8, 16 * FL_G], I32, kind="ExternalOutput")

    from contextlib import ExitStack
    with tile.TileContext(nc) as tc:
        with ExitStack() as stack:
            cst = stack.enter_context(tc.tile_pool(name="consts", bufs=1))
            sxp = stack.enter_context(tc.tile_pool(name="sxp", bufs=2))
            cbp = stack.enter_context(tc.tile_pool(name="cbp", bufs=2))
            ebp = stack.enter_context(tc.tile_pool(name="ebp", bufs=6))
            xbp = stack.enter_context(tc.tile_pool(name="xbp", bufs=2))
            xhp = stack.enter_context(tc.tile_pool(name="xhp", bufs=2))
            zwp = stack.enter_context(tc.tile_pool(name="zwp", bufs=2))
            y2p = stack.enter_context(tc.tile_pool(name="y2p", bufs=1))
            zdp = stack.enter_context(tc.tile_pool(name="zdp", bufs=3))
            zip_ = stack.enter_context(tc.tile_pool(name="zip", bufs=4))
            lp = stack.enter_context(tc.tile_pool(name="lp", bufs=2))
            ltmp = stack.enter_context(tc.tile_pool(name="ltmp", bufs=1))
            dkp = stack.enter_context(tc.tile_pool(name="dkp", bufs=6))
            glp = stack.enter_context(tc.tile_pool(name="glp", bufs=2))
            gop = stack.enter_context(tc.tile_pool(name="gop", bufs=3))
            psp = stack.enter_context(tc.tile_pool(name="ps", bufs=4, space="PSUM"))
            pslp = stack.enter_context(tc.tile_pool(name="psl", bufs=2, space="PSUM"))
            psdp = stack.enter_context(tc.tile_pool(name="psd", bufs=2, space="PSUM"))
            # ---- constants to SBUF ----
            ght = cst.tile([HS, HC], BF16)
            nc.sync.dma_start(ght[:], gh_h.ap())
            gw0 = cst.tile([WIN, HC], BF16)
            nc.sync.dma_start(gw0[:], gw0_h.ap())
            gw1 = cst.tile([WIN, HC], BF16)
            nc.sync.dma_start(gw1[:], gw1_h.ap())
            gd0 = cst.tile([WIN, HC], BF16)
            nc.sync.dma_start(gd0[:], gd0_h.ap())
            gd1 = cst.tile([WIN, HC], BF16)
            nc.sync.dma_start(gd1[:], gd1_h.ap())
            whtt = cst.tile([4, HS], BF16)
            nc.sync.dma_start(whtt[:], wht_h.ap())
            c16t = cst.tile([128, 16], F32)
            nc.sync.dma_start(c16t[:], c16_h.ap())
            tabf = cst.tile([128, N_LABELS], F32)
            nc.sync.dma_start(tabf[:], tabf_h.ap())
            tab32 = cst.tile([128, N_LABELS], I32)
            nc.sync.dma_start(tab32[:], tab32_h.ap())
            idbf = cst.tile([128, 128], BF16)
            nc.sync.dma_start(idbf[:], idbf_h.ap())

            # y2 SBUF-resident: w' halves on partitions 0..95, free (h', d)
            y2a = y2p.tile([HC, HC * D], BF16, tag="y2a")
            y2b = y2p.tile([HC, HC * D], BF16, tag="y2b")

            def emit_label_dve(s0, fc):
                """DVE-direct 16-entry packed LUT on labo cols [s0, s0+fc)."""
                lt = lp.tile([128, fc], I16, tag="lt")
                nc.scalar.dma_start(lt[:], lab_h.ap()[:, s0:s0 + fc])
                hh = ltmp.tile([128, fc], I16, tag="hh")
                sh = ltmp.tile([128, fc], I16, tag="sh")
                acc = ltmp.tile([128, fc], I16, tag="acc")
                ek = ltmp.tile([128, fc], I16, tag="ek")
                o16 = lp.tile([128, fc], I16, tag="o16")
                nc.vector.tensor_scalar(hh[:], lt[:], 1, None, A.logical_shift_right)
                nc.vector.tensor_scalar(sh[:], lt[:], 1, None, A.bitwise_and)
                nc.vector.tensor_scalar(sh[:], sh[:], 7, None, A.mult)
                nc.vector.tensor_scalar(acc[:], hh[:], 0, c16t[:, 0:1], A.is_equal, A.mult)
                for k in range(1, 16):
                    nc.vector.tensor_scalar(ek[:], hh[:], k, c16t[:, k:k + 1], A.is_equal, A.mult)
                    # disjoint-one-hot accumulate: paired 32-bit bitwise OR
                    # (2 int16 lanes per op; int32 adds would round via fp32)
                    nc.vector.tensor_tensor(acc[:].bitcast(I32), acc[:].bitcast(I32),
                                            ek[:].bitcast(I32), A.bitwise_or)
                nc.vector.tensor_tensor(acc[:], acc[:], sh[:], A.logical_shift_right)
                nc.vector.tensor_scalar(o16[:], acc[:], 127, None, A.bitwise_and)
                nc.sync.dma_start(labo_h.ap()[:, s0:s0 + fc], o16[:])

            def emit_label_pe(s0, fc):
                """PE path: 32 one-hot compares (DVE, table value folded in,
                <=99 so bf16-exact) -> identity-matmul PSUM accumulation.
                Compares are emitted LOOKAHEAD passes ahead of their matmuls
                so the in-order PE queue doesn't stall on DVE."""
                LOOKAHEAD = 4
                lt = lp.tile([128, fc], I16, tag="lt")
                nc.scalar.dma_start(lt[:], lab_h.ap()[:, s0:s0 + fc])
                ob = lp.tile([128, fc], BF16, tag="ob")
                nq = (fc + 511) // 512
                psls = []
                for _ in range(nq):
                    pslt = pslp.tile([128, 512], F32, tag="psl")
                    psls.append(pslt)
                dks = {}
                def emit_compare(k):
                    dk = dkp.tile([128, fc], BF16, tag="dk")
                    nc.vector.tensor_scalar(dk[:], lt[:], k, tabf[:, k:k + 1],
                                            A.is_equal, A.mult)
                    dks[k] = dk
                for k in range(LOOKAHEAD):
                    emit_compare(k)
                for k in range(N_LABELS):
                    if k + LOOKAHEAD < N_LABELS:
                        emit_compare(k + LOOKAHEAD)
                    dk = dks.pop(k)
                    for q in range(nq):
                        qn = min(512, fc - q * 512)
                        nc.tensor.matmul(psls[q][:, :qn], idbf[:],
                                         dk[:, q * 512:q * 512 + qn],
                                         start=(k == 0), stop=(k == N_LABELS - 1))
                for q in range(nq):
                    qn = min(512, fc - q * 512)
                    nc.vector.tensor_copy(ob[:, q * 512:q * 512 + qn], psls[q][:, :qn])
                nc.sync.dma_start(labp_h.ap()[:, s0 - FL_DVE:s0 - FL_DVE + fc], ob[:])

            def emit_label_gather(s0, fc):
                """GPSIMD ap_gather over gather-share cols [s0, s0+fc)."""
                ltg = glp.tile([128, fc], I16, tag="ltg")
                base = FL_DVE + FL_PE
                nc.sync.dma_start(ltg[:], lab_h.ap()[:, base + s0:base + s0 + fc])
                pend_g = None
                for g0 in list(range(0, fc, GCH)) + [None]:
                    if g0 is not None:
                        og = gop.tile([128, 16 * GCH], I32, tag="og")
                        nc.gpsimd.ap_gather(og[:], tab32[:], ltg[:, g0:g0 + GCH],
                                            channels=128, num_elems=N_LABELS, d=1,
                                            num_idxs=16 * GCH)
                    if pend_g is not None:
                        og_p, g0_p = pend_g
                        pstep = og_p[:].ap[0][0]
                        nc.sync.dma_start(
                            labg_h.ap()[:, 16 * (s0 + g0_p):16 * (s0 + g0_p + GCH)],
                            bass.AP(og_p.tensor, og_p[:].offset,
                                    [[pstep * 16, 8], [1, 16 * GCH]]),
                        )
                    pend_g = (og, g0) if g0 is not None else None

            ldve_chunks = [(s, min(LCH, FL_DVE - s)) for s in range(0, FL_DVE, LCH)]
            lpe_chunks = [(FL_DVE + s, min(PCH, FL_PE - s)) for s in range(0, FL_PE, PCH)]
            lg_chunks = [(s, min(GLD, FL_G - s)) for s in range(0, FL_G, GLD)]
            li = [0, 0, 0]

            def drip_labels(fd, fp, fg):
                # per-path completion targets (fractions of each chunk list)
                if _NO_LABELS:
                    return
                while li[0] < len(ldve_chunks) * fd:
                    emit_label_dve(*ldve_chunks[li[0]])
                    li[0] += 1
                while li[1] < len(lpe_chunks) * fp:
                    emit_label_pe(*lpe_chunks[li[1]])
                    li[1] += 1
                while li[2] < len(lg_chunks) * fg:
                    emit_label_gather(*lg_chunks[li[2]])
                    li[2] += 1

            # ================= stage A (software-pipelined) =================
            def phase_a1(ib):
                """load + bias matmul + exp; returns (sx, ebs)."""
                d0 = ib * DB
                sx = sxp.tile([HS, FA], BF16)
                nc.sync.dma_start(sx[:], xs_h.ap()[:, d0 * W:(d0 + DB) * W])
                cb = cbp.tile([4, FA], BF16)
                nc.sync.dma_start(cb[:], c_h.ap()[:, d0 * W:(d0 + DB) * W])
                ebs = []
                for q in range(FA // 512):
                    sl = slice(q * 512, (q + 1) * 512)
                    psb = psp.tile([HS, 512], F32, tag="ps")
                    nc.tensor.matmul(psb[:], whtt[:], cb[:, sl], start=True, stop=True)
                    eb = ebp.tile([HS, 512], BF16, tag="eb")
                    nc.scalar.activation(eb[:], psb[:], mybir.ActivationFunctionType.Exp)
                    ebs.append(eb)
                return sx, ebs

            def phase_a2(ib, sx, ebs):
                """mult, H-blur, T1, W-blur -> y2."""
                d0 = ib * DB
                xb = xbp.tile([HS, FA], BF16)
                for q in range(FA // 512):
                    sl = slice(q * 512, (q + 1) * 512)
                    nc.vector.tensor_tensor(xb[:, sl], sx[:, sl], ebs[q][:], A.mult)

                xh = xhp.tile([HC, FA], BF16)
                for q in range(FA // 512):
                    sl = slice(q * 512, (q + 1) * 512)
                    psh = psp.tile([HC, 512], F32, tag="ps")
                    nc.tensor.matmul(psh[:], ght[:], xb[:, sl], start=True, stop=True)
                    nc.scalar.copy(xh[:, sl], psh[:])

                # T1: w onto partitions, two 108-row windows; zw free = (dl, h')
                zw0 = zwp.tile([WIN, DB * HC], BF16, tag="zw0")
                zw1 = zwp.tile([WIN, DB * HC], BF16, tag="zw1")
                for half in range(2):
                    pt0 = psp.tile([WIN, 4 * HC], BF16, tag="ps")
                    pt1 = psp.tile([WIN, 4 * HC], BF16, tag="ps")
                    for t in range(4):
                        dl = half * 4 + t
                        nc.tensor.transpose(
                            pt0[:, t * HC:(t + 1) * HC],
                            xh[:, dl * W: dl * W + WIN], idbf[0:HC, 0:HC])
                        nc.tensor.transpose(
                            pt1[:, t * HC:(t + 1) * HC],
                            xh[:, dl * W + 84: dl * W + 192], idbf[0:HC, 0:HC])
                    nc.vector.tensor_copy(zw0[:, half * 4 * HC:(half + 1) * 4 * HC], pt0[:])
                    nc.vector.tensor_copy(zw1[:, half * 4 * HC:(half + 1) * 4 * HC], pt1[:])

                # banded W-blur -> y2 (SBUF), out tile m covers w' in [96m, 96m+96)
                for m, (gwm, zwm, y2m) in enumerate(((gw0, zw0, y2a), (gw1, zw1, y2b))):
                    for q in range(2):
                        sl = slice(q * 4 * HC, (q + 1) * 4 * HC)
                        psw = psp.tile([HC, 4 * HC], F32, tag="ps")
                        nc.tensor.matmul(psw[:], gwm[:], zwm[:, sl], start=True, stop=True)
                        # psw free = (dl 4, h' 96) -> y2 free (h'*D + d) at d0+q*4
                        nc.scalar.copy(
                            bass.AP(y2m.tensor,
                                    y2m[:].offset + d0 + q * 4,
                                    [[y2m[:].ap[0][0], HC], [1, 4], [D, HC]]),
                            psw[:],
                        )

            pend = None
            for ib in range(NB_A + 1 if not _NO_IMG else 0):
                cur = phase_a1(ib) if ib < NB_A else None
                if pend is not None:
                    phase_a2(ib - 1, *pend)
                    u = ib / NB_A
                    drip_labels(u * 0.26, u, u * 0.67)
                pend = cur

            # ================= stage B (software-pipelined) =================
            def phase_b1(jb):
                """T2 transposes + zd copies; returns (zd0, zd1)."""
                h0 = jb * HB
                zd0 = zdp.tile([WIN, HB * W], BF16, tag="zd0")
                zd1 = zdp.tile([WIN, HB * W], BF16, tag="zd1")
                for half in range(2):
                    pt0 = psp.tile([WIN, 4 * 2 * HC], BF16, tag="ps")
                    pt1 = psp.tile([WIN, 4 * 2 * HC], BF16, tag="ps")
                    for t in range(4):
                        hl = half * 4 + t
                        hbase = (h0 + hl) * D
                        nc.tensor.transpose(
                            pt0[:, t * 2 * HC: t * 2 * HC + HC],
                            y2a[:, hbase: hbase + WIN], idbf[0:HC, 0:HC])
                        nc.tensor.transpose(
                            pt0[:, t * 2 * HC + HC: (t + 1) * 2 * HC],
                            y2b[:, hbase: hbase + WIN], idbf[0:HC, 0:HC])
                        nc.tensor.transpose(
                            pt1[:, t * 2 * HC: t * 2 * HC + HC],
                            y2a[:, hbase + 84: hbase + 192], idbf[0:HC, 0:HC])
                        nc.tensor.transpose(
                            pt1[:, t * 2 * HC + HC: (t + 1) * 2 * HC],
                            y2b[:, hbase + 84: hbase + 192], idbf[0:HC, 0:HC])
                    nc.scalar.copy(
                        zd0[:, half * 4 * W:(half + 1) * 4 * W], pt0[:])
                    nc.scalar.copy(
                        zd1[:, half * 4 * W:(half + 1) * 4 * W], pt1[:])
                return zd0, zd1

            def phase_b2(jb, zd0, zd1):
                """banded D-blur + img out; out tile m = d' in [96m, 96m+96)."""
                h0 = jb * HB
                for m, (gdm, zdm) in enumerate(((gd0, zd0), (gd1, zd1))):
                    for q in range(3):
                        sl = slice(q * 512, (q + 1) * 512)
                        psd = psdp.tile([HC, 512], F32, tag="psd")
                        nc.tensor.matmul(psd[:], gdm[:], zdm[:, sl], start=True, stop=True)
                        zi = zip_.tile([HC, 512], BF16, tag="zi")
                        nc.vector.tensor_copy(zi[:], psd[:])
                        nc.sync.dma_start(
                            bass.AP(img_h, m * HC * HC * W + h0 * W + q * 512,
                                    [[HC * W, HC], [1, 512]]),
                            zi[:],
                        )

            pendb = None
            for jb in range(NB_B + 1 if not _NO_IMG else 0):
                curb = phase_b1(jb) if jb < NB_B else None
                if pendb is not None:
                    phase_b2(jb - 1, *pendb)
                    v = jb / NB_B
                    drip_labels(0.26 + v * 0.74, 1.0, 0.67 + v * 0.33)
                pendb = curb

            drip_labels(1.0, 1.0, 1.0)

    nc.compile()
    return nc


def _host_prep(x, small_bias, sigma01, labels, source_values, dest_values):
    Wd = _lin_weights(SMALL, D)
    Whm = _lin_weights(SMALL, H)
    Wwm = _lin_weights(SMALL, W)
    eyebf = np.eye(128, dtype=ml_dtypes.bfloat16)

    mapping = np.zeros(TABLE, np.int64)
    mapping[np.asarray(source_values, np.int64)] = np.asarray(dest_values, np.int64)
    T = mapping[:N_LABELS]
    C16 = (T[0::2] | (T[1::2] << 7)).astype(np.float32)
    c16_rep = np.broadcast_to(C16, (128, 16)).copy()
    tabf_rep = np.broadcast_to(T.astype(np.float32), (128, N_LABELS)).copy()
    tab32_rep = np.broadcast_to(T.astype(np.int32), (128, N_LABELS)).copy()

    in_maps = []
    for c in range(8):
        b, half = c // 2, c % 2
        h0 = half * HC
        hidx = np.clip(np.arange(h0 - P, h0 + HC + P), 0, H - 1)

        # x slab, h-major layout [HS, D, W] in bf16
        xs = np.ascontiguousarray(
            np.asarray(x[b, 0], np.float32)[:, hidx, :].transpose(1, 0, 2)
        ).astype(ml_dtypes.bfloat16).reshape(HS, D * W)

        sm = np.asarray(small_bias[b, 0], np.float64) * BIAS_STD
        Cydw = np.einsum("xyz,dx,wz->ydw", sm, Wd, Wwm).reshape(4, D * W)
        WhT = np.ascontiguousarray(Whm[hidx, :].T)

        g3 = _gauss_kernels(np.asarray(sigma01[b], np.float64) * MAX_SIGMA)
        Gh = _slab_toeplitz(g3[1])
        Mw = _edge_folded_toeplitz(g3[2], W)
        Md = _edge_folded_toeplitz(g3[0], D)
        Gw0 = Mw[0:WIN, 0:HC]
        Gw1 = Mw[84:192, HC:192]
        Gd0 = Md[0:WIN, 0:HC]
        Gd1 = Md[84:192, HC:192]

        lab = np.asarray(labels[b, 0][:, h0:h0 + HC, :], np.int16).reshape(128, FLAB)

        in_maps.append({
            "xs": xs,
            "cydw": Cydw.astype(ml_dtypes.bfloat16),
            "wht": WhT.astype(ml_dtypes.bfloat16),
            "gh": Gh.astype(ml_dtypes.bfloat16),
            "gw0": Gw0.astype(ml_dtypes.bfloat16),
            "gw1": Gw1.astype(ml_dtypes.bfloat16),
            "gd0": Gd0.astype(ml_dtypes.bfloat16),
            "gd1": Gd1.astype(ml_dtypes.bfloat16),
            "lab": np.ascontiguousarray(lab),
            "c16": c16_rep,
            "tabf": tabf_rep,
            "tab32": tab32_rep,
            "idbf": eyebf,
        })
    return in_maps


def kernel(x, small_bias, sigma01, labels, source_values, dest_values):
    if "nc" not in _CACHE:
        _CACHE["nc"] = _build_program()
    nc = _CACHE["nc"]

    in_maps = _host_prep(x, small_bias, sigma01, labels, source_values, dest_values)
    res = run_bass_kernel_spmd(nc, in_maps, core_ids=list(range(8)))

    img = np.empty((B, C, D, H, W), np.float32)
    labels_out = np.empty((B, C, D, H, W), np.int32)
    for c in range(8):
        b, half = c // 2, c % 2
        h0 = half * HC
        r = res.results[c]
        img[b, 0, :, h0:h0 + HC, :] = r["img"].reshape(D, HC, W).astype(np.float32)
        lo = np.empty((128, FLAB), np.int32)
        lo[:, :FL_DVE] = r["labo"].astype(np.int32)
        lo[:, FL_DVE:FL_DVE + FL_PE] = np.asarray(r["labp"], np.float32).astype(np.int32)
        # gather share: labg [8, 16*FL_G], row g holds group g's elements in
        # (slot-major, partition-interleaved) order
        lg = r["labg"].astype(np.int32).reshape(8, FL_G, 16)
        for g in range(8):
            lo[16 * g:16 * g + 16, FL_DVE + FL_PE:] = lg[g].T
        labels_out[b, 0, :, h0:h0 + HC, :] = lo.reshape(D, HC, W)
    return img, labels_out


# revision 76
# speedup vs baseline: 3.2325x; 1.0476x over previous
"""Trainium2 Bass kernel for nn_BrainGeneratorModel (bias-field corrupt + per-sample
separable Gaussian blur + label LUT remap), 8-core data/spatial parallel.

Sharding: 8 cores = (sample b in 0..3) x (H-half in 0..1). Each core processes a
[D=192, H=96(+12 halo), W=192] subvolume of one sample plus its label slice.

Per-core pipeline:
  A) stream d-batches: bias matmul (K=4) -> exp (ACT, bf16) -> x*expb (DVE)
     -> H-blur matmul (banded 120->96) -> PE transposes (w onto partitions,
     two 108-row windows) -> banded W-blur (one 108-contraction matmul per
     96-row output tile) -> y2 kept resident in SBUF as bf16 [w', (h', d)]
  B) stream h-batches from SBUF y2: PE transposes (d onto partitions, two
     108-row windows) -> banded D-blur -> img out as bf16.
  L) labels split three ways: DVE-direct 16-entry packed-int16 LUT
     (C16[h] = T[2h] | T[2h+1]<<7, one-hot compare chain with paired 32-bit
     OR accumulate), PE path (32 one-hot compares with the bf16-exact table
     value folded in, accumulated via identity matmuls in PSUM), and GPSIMD
     ap_gather (per-16-partition-group shared-index gather, host deinterleaves).
"""

import sys

for _p in ("/opt/trn_rl_repo",):
    if _p not in sys.path:
        sys.path.insert(0, _p)

import numpy as np
import ml_dtypes

import concourse.bass as bass
import concourse.mybir as mybir
import concourse.bacc as bacc
import concourse.tile as tile
from concourse.bass_utils import run_bass_kernel_spmd

F32 = mybir.dt.float32
BF16 = mybir.dt.bfloat16
I16 = mybir.dt.int16
I32 = mybir.dt.int32
A = mybir.AluOpType

B, C, D, H, W = 4, 1, 192, 192, 192
SMALL = 4
BIAS_STD = 0.7
MAX_SIGMA = 3.0
TRUNCATE = 4.0
K = 2 * int(TRUNCATE * MAX_SIGMA) + 1  # 25
P = K // 2  # 12
N_LABELS = 32
TABLE = 128

HC = 96            # interior H rows per core
HS = 120           # slab rows = HC + 2*P
DB = 8             # d-batch size (stage A)
NB_A = D // DB     # 24 batches
HB = 8             # h-batch size (stage B)
NB_B = HC // HB    # 12 batches
FA = DB * W        # 1536 stage-A free size
WIN = 108          # banded blur input window (96 + 12)
FLAB = D * HC * W // 128  # 27648 label cols per partition

# --- label split across engines (cols) ---
FL_DVE = 9600      # DVE-direct share (16-entry packed LUT)
FL_PE = 8960       # PE share (32 scaled-one-hot matmul accumulation)
FL_G = FLAB - FL_DVE - FL_PE  # 10752 -> gpsimd ap_gather share
LCH = 1920         # chunk cols for the DVE label path
PCH = 1024         # chunk cols for the PE label path
GCH = 128          # cols per ap_gather instruction (out free = 16*GCH)
GLD = 1024         # cols per gather-path input DMA

_CACHE = {}

import os as _os
_NO_LABELS = bool(int(_os.environ.get("KERN_NO_LABELS", "0")))
_NO_IMG = bool(int(_os.environ.get("KERN_NO_IMG", "0")))


def _lin_weights(n_in, n_out):
    pos = np.linspace(0.0, n_in - 1.0, n_out, dtype=np.float64)
    i0 = np.clip(np.floor(pos).astype(np.int64), 0, n_in - 2)
    f = pos - i0
    Wm = np.zeros((n_out, n_in), np.float64)
    r = np.arange(n_out)
    np.add.at(Wm, (r, i0), 1.0 - f)
    np.add.at(Wm, (r, i0 + 1), f)
    return Wm


def _gauss_kernels(sigma3):
    ar = np.arange(K, dtype=np.float64) - K // 2
    out = np.zeros((3, K), np.float64)
    for i, sg in enumerate(sigma3):
        s = max(float(sg), 1e-3)
        g = np.exp(-0.5 * ar * ar / (s * s))
        g = g / g.sum()
        if float(sg) >= 0.01:
            out[i] = g
        else:
            out[i, K // 2] = 1.0
    return out


def _edge_folded_toeplitz(g, n):
    """[n, n] matrix M with out[j] = sum_i M[i, j] * x[i], replicate padding."""
    M = np.zeros((n, n), np.float64)
    for j in range(n):
        for t in range(K):
            src = min(max(j + t - P, 0), n - 1)
            M[src, j] += g[t]
    return M


def _slab_toeplitz(g):
    """[HS, HC]: slab rows (pre-clipped by host) -> interior outputs."""
    M = np.zeros((HS, HC), np.float64)
    for j in range(HC):
        for t in range(K):
            M[j + t, j] += g[t]
    return M


def _build_program():
    nc = bacc.Bacc("TRN2", target_bir_lowering=False, debug=False)

    # ---- external inputs (per core) ----
    xs_h = nc.dram_tensor("xs", [HS, D * W], BF16, kind="ExternalInput")
    c_h = nc.dram_tensor("cydw", [4, D * W], BF16, kind="ExternalInput")
    wht_h = nc.dram_tensor("wht", [4, HS], BF16, kind="ExternalInput")
    gh_h = nc.dram_tensor("gh", [HS, HC], BF16, kind="ExternalInput")
    gw0_h = nc.dram_tensor("gw0", [WIN, HC], BF16, kind="ExternalInput")
    gw1_h = nc.dram_tensor("gw1", [WIN, HC], BF16, kind="ExternalInput")
    gd0_h = nc.dram_tensor("gd0", [WIN, HC], BF16, kind="ExternalInput")
    gd1_h = nc.dram_tensor("gd1", [WIN, HC], BF16, kind="ExternalInput")
    lab_h = nc.dram_tensor("lab", [128, FLAB], I16, kind="ExternalInput")
    c16_h = nc.dram_tensor("c16", [128, 16], F32, kind="ExternalInput")
    tabf_h = nc.dram_tensor("tabf", [128, N_LABELS], F32, kind="ExternalInput")
    tab32_h = nc.dram_tensor("tab32", [128, N_LABELS], I32, kind="ExternalInput")
    idbf_h = nc.dram_tensor("idbf", [128, 128], BF16, kind="ExternalInput")

    # ---- external outputs ----
    img_h = nc.dram_tensor("img", [D, HC, W], BF16, kind="ExternalOutput")
    labo_h = nc.dram_tensor("labo", [128, FL_DVE], I16, kind="ExternalOutput")
    labp_h = nc.dram_tensor("labp", [128, FL_PE], BF16, kind="ExternalOutput")
    labg_h = nc.dram_tensor("labg", [8, 16 * FL_G], I32, kind="ExternalOutput")

    from contextlib import ExitStack
    with tile.TileContext(nc) as tc:
        with ExitStack() as stack:
            cst = stack.enter_context(tc.tile_pool(name="consts", bufs=1))
            sxp = stack.enter_context(tc.tile_pool(name="sxp", bufs=2))
            cbp = stack.enter_context(tc.tile_pool(name="cbp", bufs=2))
            ebp = stack.enter_context(tc.tile_pool(name="ebp", bufs=6))
            xbp = stack.enter_context(tc.tile_pool(name="xbp", bufs=2))
            xhp = stack.enter_context(tc.tile_pool(name="xhp", bufs=2))
            zwp = stack.enter_context(tc.tile_pool(name="zwp", bufs=2))
            y2p = stack.enter_context(tc.tile_pool(name="y2p", bufs=1))
            zdp = stack.enter_context(tc.tile_pool(name="zdp", bufs=3))
            zip_ = stack.enter_context(tc.tile_pool(name="zip", bufs=4))
            lp = stack.enter_context(tc.tile_pool(name="lp", bufs=2))
            ltmp = stack.enter_context(tc.tile_pool(name="ltmp", bufs=1))
            dkp = stack.enter_context(tc.tile_pool(name="dkp", bufs=6))
            glp = stack.enter_context(tc.tile_pool(name="glp", bufs=2))
            gop = stack.enter_context(tc.tile_pool(name="gop", bufs=3))
            psp = stack.enter_context(tc.tile_pool(name="ps", bufs=4, space="PSUM"))
            pslp = stack.enter_context(tc.tile_pool(name="psl", bufs=2, space="PSUM"))
            psdp = stack.enter_context(tc.tile_pool(name="psd", bufs=2, space="PSUM"))
            # ---- constants to SBUF ----
            ght = cst.tile([HS, HC], BF16)
            nc.sync.dma_start(ght[:], gh_h.ap())
            gw0 = cst.tile([WIN, HC], BF16)
            nc.sync.dma_start(gw0[:], gw0_h.ap())
            gw1 = cst.tile([WIN, HC], BF16)
            nc.sync.dma_start(gw1[:], gw1_h.ap())
            gd0 = cst.tile([WIN, HC], BF16)
            nc.sync.dma_start(gd0[:], gd0_h.ap())
            gd1 = cst.tile([WIN, HC], BF16)
            nc.sync.dma_start(gd1[:], gd1_h.ap())
            whtt = cst.tile([4, HS], BF16)
            nc.sync.dma_start(whtt[:], wht_h.ap())
            c16t = cst.tile([128, 16], F32)
            nc.sync.dma_start(c16t[:], c16_h.ap())
            tabf = cst.tile([128, N_LABELS], F32)
            nc.sync.dma_start(tabf[:], tabf_h.ap())
            tab32 = cst.tile([128, N_LABELS], I32)
            nc.sync.dma_start(tab32[:], tab32_h.ap())
            idbf = cst.tile([128, 128], BF16)
            nc.sync.dma_start(idbf[:], idbf_h.ap())

            # y2 SBUF-resident: w' halves on partitions 0..95, free (h', d)
            y2a = y2p.tile([HC, HC * D], BF16, tag="y2a")
            y2b = y2p.tile([HC, HC * D], BF16, tag="y2b")

            def emit_label_dve(s0, fc):
                """DVE-direct 16-entry packed LUT on labo cols [s0, s0+fc)."""
                lt = lp.tile([128, fc], I16, tag="lt")
                nc.scalar.dma_start(lt[:], lab_h.ap()[:, s0:s0 + fc])
                hh = ltmp.tile([128, fc], I16, tag="hh")
                sh = ltmp.tile([128, fc], I16, tag="sh")
                acc = ltmp.tile([128, fc], I16, tag="acc")
                ek = ltmp.tile([128, fc], I16, tag="ek")
                o16 = lp.tile([128, fc], I16, tag="o16")
                nc.vector.tensor_scalar(hh[:], lt[:], 1, None, A.logical_shift_right)
                nc.vector.tensor_scalar(sh[:], lt[:], 1, None, A.bitwise_and)
                nc.vector.tensor_scalar(sh[:], sh[:], 7, None, A.mult)
                nc.vector.tensor_scalar(acc[:], hh[:], 0, c16t[:, 0:1], A.is_equal, A.mult)
                for k in range(1, 16):
                    nc.vector.tensor_scalar(ek[:], hh[:], k, c16t[:, k:k + 1], A.is_equal, A.mult)
                    # disjoint-one-hot accumulate: paired 32-bit bitwise OR
                    # (2 int16 lanes per op; int32 adds would round via fp32)
                    nc.vector.tensor_tensor(acc[:].bitcast(I32), acc[:].bitcast(I32),
                                            ek[:].bitcast(I32), A.bitwise_or)
                nc.vector.tensor_tensor(acc[:], acc[:], sh[:], A.logical_shift_right)
                nc.vector.tensor_scalar(o16[:], acc[:], 127, None, A.bitwise_and)
                nc.sync.dma_start(labo_h.ap()[:, s0:s0 + fc], o16[:])

            def emit_label_pe(s0, fc):
                """PE path: 32 one-hot compares (DVE, table value folded in,
                <=99 so bf16-exact) -> identity-matmul PSUM accumulation.
                Compares are emitted LOOKAHEAD passes ahead of their matmuls
                so the in-order PE queue doesn't stall on DVE."""
                LOOKAHEAD = 4
                lt = lp.tile([128, fc], I16, tag="lt")
                nc.scalar.dma_start(lt[:], lab_h.ap()[:, s0:s0 + fc])
                ob = lp.tile([128, fc], BF16, tag="ob")
                nq = (fc + 511) // 512
                psls = []
                for _ in range(nq):
                    pslt = pslp.tile([128, 512], F32, tag="psl")
                    psls.append(pslt)
                dks = {}
                def emit_compare(k):
                    dk = dkp.tile([128, fc], BF16, tag="dk")
                    nc.vector.tensor_scalar(dk[:], lt[:], k, tabf[:, k:k + 1],
                                            A.is_equal, A.mult)
                    dks[k] = dk
                for k in range(LOOKAHEAD):
                    emit_compare(k)
                for k in range(N_LABELS):
                    if k + LOOKAHEAD < N_LABELS:
                        emit_compare(k + LOOKAHEAD)
                    dk = dks.pop(k)
                    for q in range(nq):
                        qn = min(512, fc - q * 512)
                        nc.tensor.matmul(psls[q][:, :qn], idbf[:],
                                         dk[:, q * 512:q * 512 + qn],
                                         start=(k == 0), stop=(k == N_LABELS - 1))
                for q in range(nq):
                    qn = min(512, fc - q * 512)
                    nc.vector.tensor_copy(ob[:, q * 512:q * 512 + qn], psls[q][:, :qn])
                nc.sync.dma_start(labp_h.ap()[:, s0 - FL_DVE:s0 - FL_DVE + fc], ob[:])

            def emit_label_gather(s0, fc):
                """GPSIMD ap_gather over gather-share cols [s0, s0+fc)."""
                ltg = glp.tile([128, fc], I16, tag="ltg")
                base = FL_DVE + FL_PE
                nc.sync.dma_start(ltg[:], lab_h.ap()[:, base + s0:base + s0 + fc])
                pend_g = None
                for g0 in list(range(0, fc, GCH)) + [None]:
                    if g0 is not None:
                        og = gop.tile([128, 16 * GCH], I32, tag="og")
                        nc.gpsimd.ap_gather(og[:], tab32[:], ltg[:, g0:g0 + GCH],
                                            channels=128, num_elems=N_LABELS, d=1,
                                            num_idxs=16 * GCH)
                    if pend_g is not None:
                        og_p, g0_p = pend_g
                        pstep = og_p[:].ap[0][0]
                        nc.sync.dma_start(
                            labg_h.ap()[:, 16 * (s0 + g0_p):16 * (s0 + g0_p + GCH)],
                            bass.AP(og_p.tensor, og_p[:].offset,
                                    [[pstep * 16, 8], [1, 16 * GCH]]),
                        )
                    pend_g = (og, g0) if g0 is not None else None

            ldve_chunks = [(s, min(LCH, FL_DVE - s)) for s in range(0, FL_DVE, LCH)]
            lpe_chunks = [(FL_DVE + s, min(PCH, FL_PE - s)) for s in range(0, FL_PE, PCH)]
            lg_chunks = [(s, min(GLD, FL_G - s)) for s in range(0, FL_G, GLD)]
            li = [0, 0, 0]

            def drip_labels(fd, fp, fg):
                # per-path completion targets (fractions of each chunk list)
                if _NO_LABELS:
                    return
                while li[0] < len(ldve_chunks) * fd:
                    emit_label_dve(*ldve_chunks[li[0]])
                    li[0] += 1
                while li[1] < len(lpe_chunks) * fp:
                    emit_label_pe(*lpe_chunks[li[1]])
                    li[1] += 1
                while li[2] < len(lg_chunks) * fg:
                    emit_label_gather(*lg_chunks[li[2]])
                    li[2] += 1

            # ================= stage A (software-pipelined) =================
            def phase_a1(ib):
                """load + bias matmul + exp; returns (sx, ebs)."""
                d0 = ib * DB
                sx = sxp.tile([HS, FA], BF16)
                nc.sync.dma_start(sx[:], xs_h.ap()[:, d0 * W:(d0 + DB) * W])
                cb = cbp.tile([4, FA], BF16)
                nc.sync.dma_start(cb[:], c_h.ap()[:, d0 * W:(d0 + DB) * W])
                ebs = []
                for q in range(FA // 512):
                    sl = slice(q * 512, (q + 1) * 512)
                    psb = psp.tile([HS, 512], F32, tag="ps")
                    nc.tensor.matmul(psb[:], whtt[:], cb[:, sl], start=True, stop=True)
                    eb = ebp.tile([HS, 512], BF16, tag="eb")
                    nc.scalar.activation(eb[:], psb[:], mybir.ActivationFunctionType.Exp)
                    ebs.append(eb)
                return sx, ebs

            def phase_a2(ib, sx, ebs):
                """mult, H-blur, T1, W-blur -> y2."""
                d0 = ib * DB
                xb = xbp.tile([HS, FA], BF16)
                for q in range(FA // 512):
                    sl = slice(q * 512, (q + 1) * 512)
                    nc.vector.tensor_tensor(xb[:, sl], sx[:, sl], ebs[q][:], A.mult)

                xh = xhp.tile([HC, FA], BF16)
                for q in range(FA // 512):
                    sl = slice(q * 512, (q + 1) * 512)
                    psh = psp.tile([HC, 512], F32, tag="ps")
                    nc.tensor.matmul(psh[:], ght[:], xb[:, sl], start=True, stop=True)
                    nc.scalar.copy(xh[:, sl], psh[:])

                # T1: w onto partitions, two 108-row windows; zw free = (dl, h')
                zw0 = zwp.tile([WIN, DB * HC], BF16, tag="zw0")
                zw1 = zwp.tile([WIN, DB * HC], BF16, tag="zw1")
                for half in range(2):
                    pt0 = psp.tile([WIN, 4 * HC], BF16, tag="ps")
                    pt1 = psp.tile([WIN, 4 * HC], BF16, tag="ps")
                    for t in range(4):
                        dl = half * 4 + t
                        nc.tensor.transpose(
                            pt0[:, t * HC:(t + 1) * HC],
                            xh[:, dl * W: dl * W + WIN], idbf[0:HC, 0:HC])
                        nc.tensor.transpose(
                            pt1[:, t * HC:(t + 1) * HC],
                            xh[:, dl * W + 84: dl * W + 192], idbf[0:HC, 0:HC])
                    nc.vector.tensor_copy(zw0[:, half * 4 * HC:(half + 1) * 4 * HC], pt0[:])
                    nc.vector.tensor_copy(zw1[:, half * 4 * HC:(half + 1) * 4 * HC], pt1[:])

                # banded W-blur -> y2 (SBUF), out tile m covers w' in [96m, 96m+96)
                for m, (gwm, zwm, y2m) in enumerate(((gw0, zw0, y2a), (gw1, zw1, y2b))):
                    for q in range(2):
                        sl = slice(q * 4 * HC, (q + 1) * 4 * HC)
                        psw = psp.tile([HC, 4 * HC], F32, tag="ps")
                        nc.tensor.matmul(psw[:], gwm[:], zwm[:, sl], start=True, stop=True)
                        # psw free = (dl 4, h' 96) -> y2 free (h'*D + d) at d0+q*4
                        nc.scalar.copy(
                            bass.AP(y2m.tensor,
                                    y2m[:].offset + d0 + q * 4,
                                    [[y2m[:].ap[0][0], HC], [1, 4], [D, HC]]),
                            psw[:],
                        )

            pend = None
            for ib in range(NB_A + 1 if not _NO_IMG else 0):
                cur = phase_a1(ib) if ib < NB_A else None
                if pend is not None:
                    phase_a2(ib - 1, *pend)
                    u = ib / NB_A
                    drip_labels(u * 0.26, u, u * 0.67)
                pend = cur

            # ================= stage B (software-pipelined) =================
            def phase_b1(jb):
                """T2 transposes + zd copies; returns (zd0, zd1)."""
                h0 = jb * HB
                zd0 = zdp.tile([WIN, HB * W], BF16, tag="zd0")
                zd1 = zdp.tile([WIN, HB * W], BF16, tag="zd1")
                for half in range(2):
                    pt0 = psp.tile([WIN, 4 * 2 * HC], BF16, tag="ps")
                    pt1 = psp.tile([WIN, 4 * 2 * HC], BF16, tag="ps")
                    for t in range(4):
                        hl = half * 4 + t
                        hbase = (h0 + hl) * D
                        nc.tensor.transpose(
                            pt0[:, t * 2 * HC: t * 2 * HC + HC],
                            y2a[:, hbase: hbase + WIN], idbf[0:HC, 0:HC])
                        nc.tensor.transpose(
                            pt0[:, t * 2 * HC + HC: (t + 1) * 2 * HC],
                            y2b[:, hbase: hbase + WIN], idbf[0:HC, 0:HC])
                        nc.tensor.transpose(
                            pt1[:, t * 2 * HC: t * 2 * HC + HC],
                            y2a[:, hbase + 84: hbase + 192], idbf[0:HC, 0:HC])
                        nc.tensor.transpose(
                            pt1[:, t * 2 * HC + HC: (t + 1) * 2 * HC],
                            y2b[:, hbase + 84: hbase + 192], idbf[0:HC, 0:HC])
                    nc.scalar.copy(
                        zd0[:, half * 4 * W:(half + 1) * 4 * W], pt0[:])
                    nc.vector.tensor_copy(
                        zd1[:, half * 4 * W:(half + 1) * 4 * W], pt1[:])
                return zd0, zd1

            def phase_b2(jb, zd0, zd1):
                """banded D-blur + img out; out tile m = d' in [96m, 96m+96)."""
                h0 = jb * HB
                for m, (gdm, zdm) in enumerate(((gd0, zd0), (gd1, zd1))):
                    for q in range(3):
                        sl = slice(q * 512, (q + 1) * 512)
                        psd = psdp.tile([HC, 512], F32, tag="psd")
                        nc.tensor.matmul(psd[:], gdm[:], zdm[:, sl], start=True, stop=True)
                        zi = zip_.tile([HC, 512], BF16, tag="zi")
                        if q % 2 == 0:
                            nc.scalar.copy(zi[:], psd[:])
                        else:
                            nc.vector.tensor_copy(zi[:], psd[:])
                        nc.sync.dma_start(
                            bass.AP(img_h, m * HC * HC * W + h0 * W + q * 512,
                                    [[HC * W, HC], [1, 512]]),
                            zi[:],
                        )

            pendb = None
            for jb in range(NB_B + 1 if not _NO_IMG else 0):
                curb = phase_b1(jb) if jb < NB_B else None
                if pendb is not None:
                    phase_b2(jb - 1, *pendb)
                    v = jb / NB_B
                    drip_labels(0.26 + v * 0.74, 1.0, 0.67 + v * 0.33)
                pendb = curb

            drip_labels(1.0, 1.0, 1.0)

    nc.compile()
    return nc


def _host_prep(x, small_bias, sigma01, labels, source_values, dest_values):
    Wd = _lin_weights(SMALL, D)
    Whm = _lin_weights(SMALL, H)
    Wwm = _lin_weights(SMALL, W)
    eyebf = np.eye(128, dtype=ml_dtypes.bfloat16)

    mapping = np.zeros(TABLE, np.int64)
    mapping[np.asarray(source_values, np.int64)] = np.asarray(dest_values, np.int64)
    T = mapping[:N_LABELS]
    C16 = (T[0::2] | (T[1::2] << 7)).astype(np.float32)
    c16_rep = np.broadcast_to(C16, (128, 16)).copy()
    tabf_rep = np.broadcast_to(T.astype(np.float32), (128, N_LABELS)).copy()
    tab32_rep = np.broadcast_to(T.astype(np.int32), (128, N_LABELS)).copy()

    in_maps = []
    for c in range(8):
        b, half = c // 2, c % 2
        h0 = half * HC
        hidx = np.clip(np.arange(h0 - P, h0 + HC + P), 0, H - 1)

        # x slab, h-major layout [HS, D, W] in bf16
        xs = np.ascontiguousarray(
            np.asarray(x[b, 0], np.float32)[:, hidx, :].transpose(1, 0, 2)
        ).astype(ml_dtypes.bfloat16).reshape(HS, D * W)

        sm = np.asarray(small_bias[b, 0], np.float64) * BIAS_STD
        Cydw = np.einsum("xyz,dx,wz->ydw", sm, Wd, Wwm).reshape(4, D * W)
        WhT = np.ascontiguousarray(Whm[hidx, :].T)

        g3 = _gauss_kernels(np.asarray(sigma01[b], np.float64) * MAX_SIGMA)
        Gh = _slab_toeplitz(g3[1])
        Mw = _edge_folded_toeplitz(g3[2], W)
        Md = _edge_folded_toeplitz(g3[0], D)
        Gw0 = Mw[0:WIN, 0:HC]
        Gw1 = Mw[84:192, HC:192]
        Gd0 = Md[0:WIN, 0:HC]
        Gd1 = Md[84:192, HC:192]

        lab = np.asarray(labels[b, 0][:, h0:h0 + HC, :], np.int16).reshape(128, FLAB)

        in_maps.append({
            "xs": xs,
            "cydw": Cydw.astype(ml_dtypes.bfloat16),
            "wht": WhT.astype(ml_dtypes.bfloat16),
            "gh": Gh.astype(ml_dtypes.bfloat16),
            "gw0": Gw0.astype(ml_dtypes.bfloat16),
            "gw1": Gw1.astype(ml_dtypes.bfloat16),
            "gd0": Gd0.astype(ml_dtypes.bfloat16),
            "gd1": Gd1.astype(ml_dtypes.bfloat16),
            "lab": np.ascontiguousarray(lab),
            "c16": c16_rep,
            "tabf": tabf_rep,
            "tab32": tab32_rep,
            "idbf": eyebf,
        })
    return in_maps


def kernel(x, small_bias, sigma01, labels, source_values, dest_values):
    if "nc" not in _CACHE:
        _CACHE["nc"] = _build_program()
    nc = _CACHE["nc"]

    in_maps = _host_prep(x, small_bias, sigma01, labels, source_values, dest_values)
    res = run_bass_kernel_spmd(nc, in_maps, core_ids=list(range(8)))

    img = np.empty((B, C, D, H, W), np.float32)
    labels_out = np.empty((B, C, D, H, W), np.int32)
    for c in range(8):
        b, half = c // 2, c % 2
        h0 = half * HC
        r = res.results[c]
        img[b, 0, :, h0:h0 + HC, :] = r["img"].reshape(D, HC, W).astype(np.float32)
        lo = np.empty((128, FLAB), np.int32)
        lo[:, :FL_DVE] = r["labo"].astype(np.int32)
        lo[:, FL_DVE:FL_DVE + FL_PE] = np.asarray(r["labp"], np.float32).astype(np.int32)
        # gather share: labg [8, 16*FL_G], row g holds group g's elements in
        # (slot-major, partition-interleaved) order
        lg = r["labg"].astype(np.int32).reshape(8, FL_G, 16)
        for g in range(8):
            lo[16 * g:16 * g + 16, FL_DVE + FL_PE:] = lg[g].T
        labels_out[b, 0, :, h0:h0 + HC, :] = lo.reshape(D, HC, W)
    return img, labels_out


# revision 79
# speedup vs baseline: 3.3239x; 1.0283x over previous
"""Trainium2 Bass kernel for nn_BrainGeneratorModel (bias-field corrupt + per-sample
separable Gaussian blur + label LUT remap), 8-core data/spatial parallel.

Sharding: 8 cores = (sample b in 0..3) x (H-half in 0..1). Each core processes a
[D=192, H=96(+12 halo), W=192] subvolume of one sample plus its label slice.

Per-core pipeline:
  A) stream d-batches: bias matmul (K=4) -> exp (ACT, bf16) -> x*expb (DVE)
     -> H-blur matmul (banded 120->96) -> PE transposes (w onto partitions,
     two 108-row windows) -> banded W-blur (one 108-contraction matmul per
     96-row output tile) -> y2 kept resident in SBUF as bf16 [w', (h', d)]
  B) stream h-batches from SBUF y2: PE transposes (d onto partitions, two
     108-row windows) -> banded D-blur -> img out as bf16.
  L) labels split three ways: DVE-direct 16-entry packed-int16 LUT
     (C16[h] = T[2h] | T[2h+1]<<7, one-hot compare chain with paired 32-bit
     OR accumulate), PE path (32 one-hot compares with the bf16-exact table
     value folded in, accumulated via identity matmuls in PSUM), and GPSIMD
     ap_gather (per-16-partition-group shared-index gather, host deinterleaves).
"""

import sys

for _p in ("/opt/trn_rl_repo",):
    if _p not in sys.path:
        sys.path.insert(0, _p)

import numpy as np
import ml_dtypes

import concourse.bass as bass
import concourse.mybir as mybir
import concourse.bacc as bacc
import concourse.tile as tile
from concourse.bass_utils import run_bass_kernel_spmd

F32 = mybir.dt.float32
BF16 = mybir.dt.bfloat16
I16 = mybir.dt.int16
I32 = mybir.dt.int32
A = mybir.AluOpType

B, C, D, H, W = 4, 1, 192, 192, 192
SMALL = 4
BIAS_STD = 0.7
MAX_SIGMA = 3.0
TRUNCATE = 4.0
K = 2 * int(TRUNCATE * MAX_SIGMA) + 1  # 25
P = K // 2  # 12
N_LABELS = 32
TABLE = 128

HC = 96            # interior H rows per core
HS = 120           # slab rows = HC + 2*P
DB = 8             # d-batch size (stage A)
NB_A = D // DB     # 24 batches
HB = 8             # h-batch size (stage B)
NB_B = HC // HB    # 12 batches
FA = DB * W        # 1536 stage-A free size
WIN = 108          # banded blur input window (96 + 12)
FLAB = D * HC * W // 128  # 27648 label cols per partition

# --- label split across engines (cols) ---
FL_DVE = 9600      # DVE-direct share (16-entry packed LUT)
FL_PE = 8960       # PE share (32 scaled-one-hot matmul accumulation)
FL_G = FLAB - FL_DVE - FL_PE  # 10752 -> gpsimd ap_gather share
LCH = 1920         # chunk cols for the DVE label path
PCH = 1024         # chunk cols for the PE label path
GCH = 128          # cols per ap_gather instruction (out free = 16*GCH)
GLD = 1024         # cols per gather-path input DMA

_CACHE = {}

import os as _os
_NO_LABELS = bool(int(_os.environ.get("KERN_NO_LABELS", "0")))
_NO_IMG = bool(int(_os.environ.get("KERN_NO_IMG", "0")))


def _lin_weights(n_in, n_out):
    pos = np.linspace(0.0, n_in - 1.0, n_out, dtype=np.float64)
    i0 = np.clip(np.floor(pos).astype(np.int64), 0, n_in - 2)
    f = pos - i0
    Wm = np.zeros((n_out, n_in), np.float64)
    r = np.arange(n_out)
    np.add.at(Wm, (r, i0), 1.0 - f)
    np.add.at(Wm, (r, i0 + 1), f)
    return Wm


def _gauss_kernels(sigma3):
    ar = np.arange(K, dtype=np.float64) - K // 2
    out = np.zeros((3, K), np.float64)
    for i, sg in enumerate(sigma3):
        s = max(float(sg), 1e-3)
        g = np.exp(-0.5 * ar * ar / (s * s))
        g = g / g.sum()
        if float(sg) >= 0.01:
            out[i] = g
        else:
            out[i, K // 2] = 1.0
    return out


def _edge_folded_toeplitz(g, n):
    """[n, n] matrix M with out[j] = sum_i M[i, j] * x[i], replicate padding."""
    M = np.zeros((n, n), np.float64)
    for j in range(n):
        for t in range(K):
            src = min(max(j + t - P, 0), n - 1)
            M[src, j] += g[t]
    return M


def _slab_toeplitz(g):
    """[HS, HC]: slab rows (pre-clipped by host) -> interior outputs."""
    M = np.zeros((HS, HC), np.float64)
    for j in range(HC):
        for t in range(K):
            M[j + t, j] += g[t]
    return M


def _build_program():
    nc = bacc.Bacc("TRN2", target_bir_lowering=False, debug=False)

    # ---- external inputs (per core) ----
    xs_h = nc.dram_tensor("xs", [HS, D * W], BF16, kind="ExternalInput")
    c_h = nc.dram_tensor("cydw", [4, D * W], BF16, kind="ExternalInput")
    wht_h = nc.dram_tensor("wht", [4, HS], BF16, kind="ExternalInput")
    gh_h = nc.dram_tensor("gh", [HS, HC], BF16, kind="ExternalInput")
    gw0_h = nc.dram_tensor("gw0", [WIN, HC], BF16, kind="ExternalInput")
    gw1_h = nc.dram_tensor("gw1", [WIN, HC], BF16, kind="ExternalInput")
    gd0_h = nc.dram_tensor("gd0", [WIN, HC], BF16, kind="ExternalInput")
    gd1_h = nc.dram_tensor("gd1", [WIN, HC], BF16, kind="ExternalInput")
    lab_h = nc.dram_tensor("lab", [128, FLAB], I16, kind="ExternalInput")
    c16_h = nc.dram_tensor("c16", [128, 16], F32, kind="ExternalInput")
    tabf_h = nc.dram_tensor("tabf", [128, N_LABELS], F32, kind="ExternalInput")
    tab32_h = nc.dram_tensor("tab32", [128, N_LABELS], I32, kind="ExternalInput")
    idbf_h = nc.dram_tensor("idbf", [128, 128], BF16, kind="ExternalInput")

    # ---- external outputs ----
    img_h = nc.dram_tensor("img", [D, HC, W], BF16, kind="ExternalOutput")
    labo_h = nc.dram_tensor("labo", [128, FL_DVE], I16, kind="ExternalOutput")
    labp_h = nc.dram_tensor("labp", [128, FL_PE], BF16, kind="ExternalOutput")
    labg_h = nc.dram_tensor("labg", [8, 16 * FL_G], I32, kind="ExternalOutput")

    from contextlib import ExitStack
    with tile.TileContext(nc) as tc:
        with ExitStack() as stack:
            cst = stack.enter_context(tc.tile_pool(name="consts", bufs=1))
            sxp = stack.enter_context(tc.tile_pool(name="sxp", bufs=2))
            cbp = stack.enter_context(tc.tile_pool(name="cbp", bufs=2))
            ebp = stack.enter_context(tc.tile_pool(name="ebp", bufs=6))
            xbp = stack.enter_context(tc.tile_pool(name="xbp", bufs=2))
            xhp = stack.enter_context(tc.tile_pool(name="xhp", bufs=2))
            zwp = stack.enter_context(tc.tile_pool(name="zwp", bufs=2))
            y2p = stack.enter_context(tc.tile_pool(name="y2p", bufs=1))
            zdp = stack.enter_context(tc.tile_pool(name="zdp", bufs=3))
            zip_ = stack.enter_context(tc.tile_pool(name="zip", bufs=4))
            lp = stack.enter_context(tc.tile_pool(name="lp", bufs=2))
            ltmp = stack.enter_context(tc.tile_pool(name="ltmp", bufs=1))
            dkp = stack.enter_context(tc.tile_pool(name="dkp", bufs=6))
            glp = stack.enter_context(tc.tile_pool(name="glp", bufs=2))
            gop = stack.enter_context(tc.tile_pool(name="gop", bufs=3))
            psp = stack.enter_context(tc.tile_pool(name="ps", bufs=4, space="PSUM"))
            pslp = stack.enter_context(tc.tile_pool(name="psl", bufs=2, space="PSUM"))
            psdp = stack.enter_context(tc.tile_pool(name="psd", bufs=2, space="PSUM"))
            # ---- constants to SBUF ----
            ght = cst.tile([HS, HC], BF16)
            nc.sync.dma_start(ght[:], gh_h.ap())
            gw0 = cst.tile([WIN, HC], BF16)
            nc.sync.dma_start(gw0[:], gw0_h.ap())
            gw1 = cst.tile([WIN, HC], BF16)
            nc.sync.dma_start(gw1[:], gw1_h.ap())
            gd0 = cst.tile([WIN, HC], BF16)
            nc.sync.dma_start(gd0[:], gd0_h.ap())
            gd1 = cst.tile([WIN, HC], BF16)
            nc.sync.dma_start(gd1[:], gd1_h.ap())
            whtt = cst.tile([4, HS], BF16)
            nc.sync.dma_start(whtt[:], wht_h.ap())
            c16t = cst.tile([128, 16], F32)
            nc.sync.dma_start(c16t[:], c16_h.ap())
            tabf = cst.tile([128, N_LABELS], F32)
            nc.sync.dma_start(tabf[:], tabf_h.ap())
            tab32 = cst.tile([128, N_LABELS], I32)
            nc.sync.dma_start(tab32[:], tab32_h.ap())
            idbf = cst.tile([128, 128], BF16)
            nc.sync.dma_start(idbf[:], idbf_h.ap())

            # y2 SBUF-resident: w' halves on partitions 0..95, free (h', d)
            y2a = y2p.tile([HC, HC * D], BF16, tag="y2a")
            y2b = y2p.tile([HC, HC * D], BF16, tag="y2b")

            def emit_label_dve(s0, fc):
                """DVE-direct 16-entry packed LUT on labo cols [s0, s0+fc)."""
                lt = lp.tile([128, fc], I16, tag="lt")
                nc.scalar.dma_start(lt[:], lab_h.ap()[:, s0:s0 + fc])
                hh = ltmp.tile([128, fc], I16, tag="hh")
                sh = ltmp.tile([128, fc], I16, tag="sh")
                acc = ltmp.tile([128, fc], I16, tag="acc")
                ek = ltmp.tile([128, fc], I16, tag="ek")
                o16 = lp.tile([128, fc], I16, tag="o16")
                nc.vector.tensor_scalar(hh[:], lt[:], 1, None, A.logical_shift_right)
                nc.vector.tensor_scalar(sh[:], lt[:], 1, None, A.bitwise_and)
                nc.vector.tensor_scalar(sh[:], sh[:], 7, None, A.mult)
                nc.vector.tensor_scalar(acc[:], hh[:], 0, c16t[:, 0:1], A.is_equal, A.mult)
                for k in range(1, 16):
                    nc.vector.tensor_scalar(ek[:], hh[:], k, c16t[:, k:k + 1], A.is_equal, A.mult)
                    # disjoint-one-hot accumulate: paired 32-bit bitwise OR
                    # (2 int16 lanes per op; int32 adds would round via fp32)
                    nc.vector.tensor_tensor(acc[:].bitcast(I32), acc[:].bitcast(I32),
                                            ek[:].bitcast(I32), A.bitwise_or)
                nc.vector.tensor_tensor(acc[:], acc[:], sh[:], A.logical_shift_right)
                nc.vector.tensor_scalar(o16[:], acc[:], 127, None, A.bitwise_and)
                nc.sync.dma_start(labo_h.ap()[:, s0:s0 + fc], o16[:])

            def emit_label_pe(s0, fc):
                """PE path: 32 one-hot compares (DVE, table value folded in,
                <=99 so bf16-exact) -> identity-matmul PSUM accumulation.
                Compares are emitted LOOKAHEAD passes ahead of their matmuls
                so the in-order PE queue doesn't stall on DVE."""
                LOOKAHEAD = 4
                lt = lp.tile([128, fc], I16, tag="lt")
                nc.scalar.dma_start(lt[:], lab_h.ap()[:, s0:s0 + fc])
                ob = lp.tile([128, fc], BF16, tag="ob")
                nq = (fc + 511) // 512
                psls = []
                for _ in range(nq):
                    pslt = pslp.tile([128, 512], F32, tag="psl")
                    psls.append(pslt)
                dks = {}
                def emit_compare(k):
                    dk = dkp.tile([128, fc], BF16, tag="dk")
                    nc.vector.tensor_scalar(dk[:], lt[:], k, tabf[:, k:k + 1],
                                            A.is_equal, A.mult)
                    dks[k] = dk
                for k in range(LOOKAHEAD):
                    emit_compare(k)
                for k in range(N_LABELS):
                    if k + LOOKAHEAD < N_LABELS:
                        emit_compare(k + LOOKAHEAD)
                    dk = dks.pop(k)
                    for q in range(nq):
                        qn = min(512, fc - q * 512)
                        nc.tensor.matmul(psls[q][:, :qn], idbf[:],
                                         dk[:, q * 512:q * 512 + qn],
                                         start=(k == 0), stop=(k == N_LABELS - 1))
                for q in range(nq):
                    qn = min(512, fc - q * 512)
                    nc.vector.tensor_copy(ob[:, q * 512:q * 512 + qn], psls[q][:, :qn])
                nc.sync.dma_start(labp_h.ap()[:, s0 - FL_DVE:s0 - FL_DVE + fc], ob[:])

            def emit_label_gather(s0, fc):
                """GPSIMD ap_gather over gather-share cols [s0, s0+fc)."""
                ltg = glp.tile([128, fc], I16, tag="ltg")
                base = FL_DVE + FL_PE
                nc.sync.dma_start(ltg[:], lab_h.ap()[:, base + s0:base + s0 + fc])
                pend_g = None
                for g0 in list(range(0, fc, GCH)) + [None]:
                    if g0 is not None:
                        og = gop.tile([128, 16 * GCH], I32, tag="og")
                        nc.gpsimd.ap_gather(og[:], tab32[:], ltg[:, g0:g0 + GCH],
                                            channels=128, num_elems=N_LABELS, d=1,
                                            num_idxs=16 * GCH)
                    if pend_g is not None:
                        og_p, g0_p = pend_g
                        pstep = og_p[:].ap[0][0]
                        nc.sync.dma_start(
                            labg_h.ap()[:, 16 * (s0 + g0_p):16 * (s0 + g0_p + GCH)],
                            bass.AP(og_p.tensor, og_p[:].offset,
                                    [[pstep * 16, 8], [1, 16 * GCH]]),
                        )
                    pend_g = (og, g0) if g0 is not None else None

            ldve_chunks = [(s, min(LCH, FL_DVE - s)) for s in range(0, FL_DVE, LCH)]
            lpe_chunks = [(FL_DVE + s, min(PCH, FL_PE - s)) for s in range(0, FL_PE, PCH)]
            lg_chunks = [(s, min(GLD, FL_G - s)) for s in range(0, FL_G, GLD)]
            li = [0, 0, 0]

            def drip_labels(fd, fp, fg):
                # per-path completion targets (fractions of each chunk list)
                if _NO_LABELS:
                    return
                while li[0] < len(ldve_chunks) * fd:
                    emit_label_dve(*ldve_chunks[li[0]])
                    li[0] += 1
                while li[1] < len(lpe_chunks) * fp:
                    emit_label_pe(*lpe_chunks[li[1]])
                    li[1] += 1
                while li[2] < len(lg_chunks) * fg:
                    emit_label_gather(*lg_chunks[li[2]])
                    li[2] += 1

            # ================= stage A (software-pipelined) =================
            def phase_a1(ib):
                """load + bias matmul + exp; returns (sx, ebs)."""
                d0 = ib * DB
                sx = sxp.tile([HS, FA], BF16)
                nc.sync.dma_start(sx[:], xs_h.ap()[:, d0 * W:(d0 + DB) * W])
                cb = cbp.tile([4, FA], BF16)
                nc.sync.dma_start(cb[:], c_h.ap()[:, d0 * W:(d0 + DB) * W])
                ebs = []
                for q in range(FA // 512):
                    sl = slice(q * 512, (q + 1) * 512)
                    psb = psp.tile([HS, 512], F32, tag="ps")
                    nc.tensor.matmul(psb[:], whtt[:], cb[:, sl], start=True, stop=True)
                    eb = ebp.tile([HS, 512], BF16, tag="eb")
                    nc.scalar.activation(eb[:], psb[:], mybir.ActivationFunctionType.Exp)
                    ebs.append(eb)
                return sx, ebs

            def phase_a2(ib, sx, ebs):
                """mult, H-blur, T1, W-blur -> y2."""
                d0 = ib * DB
                xb = xbp.tile([HS, FA], BF16)
                for q in range(FA // 512):
                    sl = slice(q * 512, (q + 1) * 512)
                    nc.vector.tensor_tensor(xb[:, sl], sx[:, sl], ebs[q][:], A.mult)

                xh = xhp.tile([HC, FA], BF16)
                for q in range(FA // 512):
                    sl = slice(q * 512, (q + 1) * 512)
                    psh = psp.tile([HC, 512], F32, tag="ps")
                    nc.tensor.matmul(psh[:], ght[:], xb[:, sl], start=True, stop=True)
                    nc.scalar.copy(xh[:, sl], psh[:])

                # T1: w onto partitions, two 108-row windows; zw free = (dl, h')
                zw0 = zwp.tile([WIN, DB * HC], BF16, tag="zw0")
                zw1 = zwp.tile([WIN, DB * HC], BF16, tag="zw1")
                for half in range(2):
                    pt0 = psp.tile([WIN, 4 * HC], BF16, tag="ps")
                    pt1 = psp.tile([WIN, 4 * HC], BF16, tag="ps")
                    for t in range(4):
                        dl = half * 4 + t
                        nc.tensor.transpose(
                            pt0[:, t * HC:(t + 1) * HC],
                            xh[:, dl * W: dl * W + WIN], idbf[0:HC, 0:HC])
                        nc.tensor.transpose(
                            pt1[:, t * HC:(t + 1) * HC],
                            xh[:, dl * W + 84: dl * W + 192], idbf[0:HC, 0:HC])
                    nc.vector.tensor_copy(zw0[:, half * 4 * HC:(half + 1) * 4 * HC], pt0[:])
                    nc.scalar.copy(zw1[:, half * 4 * HC:(half + 1) * 4 * HC], pt1[:])

                # banded W-blur -> y2 (SBUF), out tile m covers w' in [96m, 96m+96)
                for m, (gwm, zwm, y2m) in enumerate(((gw0, zw0, y2a), (gw1, zw1, y2b))):
                    for q in range(2):
                        sl = slice(q * 4 * HC, (q + 1) * 4 * HC)
                        psw = psp.tile([HC, 4 * HC], F32, tag="ps")
                        nc.tensor.matmul(psw[:], gwm[:], zwm[:, sl], start=True, stop=True)
                        # psw free = (dl 4, h' 96) -> y2 free (h'*D + d) at d0+q*4
                        nc.scalar.copy(
                            bass.AP(y2m.tensor,
                                    y2m[:].offset + d0 + q * 4,
                                    [[y2m[:].ap[0][0], HC], [1, 4], [D, HC]]),
                            psw[:],
                        )

            pend = None
            for ib in range(NB_A + 1 if not _NO_IMG else 0):
                cur = phase_a1(ib) if ib < NB_A else None
                if pend is not None:
                    phase_a2(ib - 1, *pend)
                    u = ib / NB_A
                    drip_labels(u * 0.26, u, u * 0.67)
                pend = cur

            # ================= stage B (software-pipelined) =================
            def phase_b1(jb):
                """T2 transposes + zd copies; returns (zd0, zd1)."""
                h0 = jb * HB
                zd0 = zdp.tile([WIN, HB * W], BF16, tag="zd0")
                zd1 = zdp.tile([WIN, HB * W], BF16, tag="zd1")
                for half in range(2):
                    pt0 = psp.tile([WIN, 4 * 2 * HC], BF16, tag="ps")
                    pt1 = psp.tile([WIN, 4 * 2 * HC], BF16, tag="ps")
                    for t in range(4):
                        hl = half * 4 + t
                        hbase = (h0 + hl) * D
                        nc.tensor.transpose(
                            pt0[:, t * 2 * HC: t * 2 * HC + HC],
                            y2a[:, hbase: hbase + WIN], idbf[0:HC, 0:HC])
                        nc.tensor.transpose(
                            pt0[:, t * 2 * HC + HC: (t + 1) * 2 * HC],
                            y2b[:, hbase: hbase + WIN], idbf[0:HC, 0:HC])
                        nc.tensor.transpose(
                            pt1[:, t * 2 * HC: t * 2 * HC + HC],
                            y2a[:, hbase + 84: hbase + 192], idbf[0:HC, 0:HC])
                        nc.tensor.transpose(
                            pt1[:, t * 2 * HC + HC: (t + 1) * 2 * HC],
                            y2b[:, hbase + 84: hbase + 192], idbf[0:HC, 0:HC])
                    nc.scalar.copy(
                        zd0[:, half * 4 * W:(half + 1) * 4 * W], pt0[:])
                    nc.scalar.copy(
                        zd1[:, half * 4 * W:(half + 1) * 4 * W], pt1[:])
                return zd0, zd1

            def phase_b2(jb, zd0, zd1):
                """banded D-blur + img out; out tile m = d' in [96m, 96m+96)."""
                h0 = jb * HB
                for m, (gdm, zdm) in enumerate(((gd0, zd0), (gd1, zd1))):
                    for q in range(3):
                        sl = slice(q * 512, (q + 1) * 512)
                        psd = psdp.tile([HC, 512], F32, tag="psd")
                        nc.tensor.matmul(psd[:], gdm[:], zdm[:, sl], start=True, stop=True)
                        zi = zip_.tile([HC, 512], BF16, tag="zi")
                        if q % 2 == 0:
                            nc.scalar.copy(zi[:], psd[:])
                        else:
                            nc.vector.tensor_copy(zi[:], psd[:])
                        nc.sync.dma_start(
                            bass.AP(img_h, m * HC * HC * W + h0 * W + q * 512,
                                    [[HC * W, HC], [1, 512]]),
                            zi[:],
                        )

            pendb = None
            for jb in range(NB_B + 1 if not _NO_IMG else 0):
                curb = phase_b1(jb) if jb < NB_B else None
                if pendb is not None:
                    phase_b2(jb - 1, *pendb)
                    v = jb / NB_B
                    drip_labels(0.26 + v * 0.74, 1.0, 0.67 + v * 0.33)
                pendb = curb

            drip_labels(1.0, 1.0, 1.0)

    nc.compile()
    return nc


def _host_prep(x, small_bias, sigma01, labels, source_values, dest_values):
    Wd = _lin_weights(SMALL, D)
    Whm = _lin_weights(SMALL, H)
    Wwm = _lin_weights(SMALL, W)
    eyebf = np.eye(128, dtype=ml_dtypes.bfloat16)

    mapping = np.zeros(TABLE, np.int64)
    mapping[np.asarray(source_values, np.int64)] = np.asarray(dest_values, np.int64)
    T = mapping[:N_LABELS]
    C16 = (T[0::2] | (T[1::2] << 7)).astype(np.float32)
    c16_rep = np.broadcast_to(C16, (128, 16)).copy()
    tabf_rep = np.broadcast_to(T.astype(np.float32), (128, N_LABELS)).copy()
    tab32_rep = np.broadcast_to(T.astype(np.int32), (128, N_LABELS)).copy()

    in_maps = []
    for c in range(8):
        b, half = c // 2, c % 2
        h0 = half * HC
        hidx = np.clip(np.arange(h0 - P, h0 + HC + P), 0, H - 1)

        # x slab, h-major layout [HS, D, W] in bf16
        xs = np.ascontiguousarray(
            np.asarray(x[b, 0], np.float32)[:, hidx, :].transpose(1, 0, 2)
        ).astype(ml_dtypes.bfloat16).reshape(HS, D * W)

        sm = np.asarray(small_bias[b, 0], np.float64) * BIAS_STD
        Cydw = np.einsum("xyz,dx,wz->ydw", sm, Wd, Wwm).reshape(4, D * W)
        WhT = np.ascontiguousarray(Whm[hidx, :].T)

        g3 = _gauss_kernels(np.asarray(sigma01[b], np.float64) * MAX_SIGMA)
        Gh = _slab_toeplitz(g3[1])
        Mw = _edge_folded_toeplitz(g3[2], W)
        Md = _edge_folded_toeplitz(g3[0], D)
        Gw0 = Mw[0:WIN, 0:HC]
        Gw1 = Mw[84:192, HC:192]
        Gd0 = Md[0:WIN, 0:HC]
        Gd1 = Md[84:192, HC:192]

        lab = np.asarray(labels[b, 0][:, h0:h0 + HC, :], np.int16).reshape(128, FLAB)

        in_maps.append({
            "xs": xs,
            "cydw": Cydw.astype(ml_dtypes.bfloat16),
            "wht": WhT.astype(ml_dtypes.bfloat16),
            "gh": Gh.astype(ml_dtypes.bfloat16),
            "gw0": Gw0.astype(ml_dtypes.bfloat16),
            "gw1": Gw1.astype(ml_dtypes.bfloat16),
            "gd0": Gd0.astype(ml_dtypes.bfloat16),
            "gd1": Gd1.astype(ml_dtypes.bfloat16),
            "lab": np.ascontiguousarray(lab),
            "c16": c16_rep,
            "tabf": tabf_rep,
            "tab32": tab32_rep,
            "idbf": eyebf,
        })
    return in_maps


def kernel(x, small_bias, sigma01, labels, source_values, dest_values):
    if "nc" not in _CACHE:
        _CACHE["nc"] = _build_program()
    nc = _CACHE["nc"]

    in_maps = _host_prep(x, small_bias, sigma01, labels, source_values, dest_values)
    res = run_bass_kernel_spmd(nc, in_maps, core_ids=list(range(8)))

    img = np.empty((B, C, D, H, W), np.float32)
    labels_out = np.empty((B, C, D, H, W), np.int32)
    for c in range(8):
        b, half = c // 2, c % 2
        h0 = half * HC
        r = res.results[c]
        img[b, 0, :, h0:h0 + HC, :] = r["img"].reshape(D, HC, W).astype(np.float32)
        lo = np.empty((128, FLAB), np.int32)
        lo[:, :FL_DVE] = r["labo"].astype(np.int32)
        lo[:, FL_DVE:FL_DVE + FL_PE] = np.asarray(r["labp"], np.float32).astype(np.int32)
        # gather share: labg [8, 16*FL_G], row g holds group g's elements in
        # (slot-major, partition-interleaved) order
        lg = r["labg"].astype(np.int32).reshape(8, FL_G, 16)
        for g in range(8):
            lo[16 * g:16 * g + 16, FL_DVE + FL_PE:] = lg[g].T
        labels_out[b, 0, :, h0:h0 + HC, :] = lo.reshape(D, HC, W)
    return img, labels_out


# revision 92
# speedup vs baseline: 3.3314x; 1.0022x over previous
"""Trainium2 Bass kernel for nn_BrainGeneratorModel (bias-field corrupt + per-sample
separable Gaussian blur + label LUT remap), 8-core data/spatial parallel.

Sharding: 8 cores = (sample b in 0..3) x (H-half in 0..1). Each core processes a
[D=192, H=96(+12 halo), W=192] subvolume of one sample plus its label slice.

Per-core pipeline:
  A) stream d-batches: bias matmul (K=4) -> exp (ACT, bf16) -> x*expb (DVE)
     -> H-blur matmul (banded 120->96) -> PE transposes (w onto partitions,
     two 108-row windows) -> banded W-blur (one 108-contraction matmul per
     96-row output tile) -> y2 kept resident in SBUF as bf16 [w', (h', d)]
  B) stream h-batches from SBUF y2: PE transposes (d onto partitions, two
     108-row windows) -> banded D-blur -> img out as bf16.
  L) labels split three ways: DVE-direct 16-entry packed-int16 LUT
     (C16[h] = T[2h] | T[2h+1]<<7, one-hot compare chain with paired 32-bit
     OR accumulate), PE path (32 one-hot compares with the bf16-exact table
     value folded in, accumulated via identity matmuls in PSUM), and GPSIMD
     ap_gather (per-16-partition-group shared-index gather, host deinterleaves).
"""

import sys

for _p in ("/opt/trn_rl_repo",):
    if _p not in sys.path:
        sys.path.insert(0, _p)

import numpy as np
import ml_dtypes

import concourse.bass as bass
import concourse.mybir as mybir
import concourse.bacc as bacc
import concourse.tile as tile
from concourse.bass_utils import run_bass_kernel_spmd

F32 = mybir.dt.float32
BF16 = mybir.dt.bfloat16
I16 = mybir.dt.int16
I32 = mybir.dt.int32
A = mybir.AluOpType

B, C, D, H, W = 4, 1, 192, 192, 192
SMALL = 4
BIAS_STD = 0.7
MAX_SIGMA = 3.0
TRUNCATE = 4.0
K = 2 * int(TRUNCATE * MAX_SIGMA) + 1  # 25
P = K // 2  # 12
N_LABELS = 32
TABLE = 128

HC = 96            # interior H rows per core
HS = 120           # slab rows = HC + 2*P
DB = 8             # d-batch size (stage A)
NB_A = D // DB     # 24 batches
HB = 8             # h-batch size (stage B)
NB_B = HC // HB    # 12 batches
FA = DB * W        # 1536 stage-A free size
WIN = 108          # banded blur input window (96 + 12)
FLAB = D * HC * W // 128  # 27648 label cols per partition

# --- label split across engines (cols) ---
FL_DVE = 9600      # DVE-direct share (16-entry packed LUT)
FL_PE = 8960       # PE share (32 scaled-one-hot matmul accumulation)
FL_G = FLAB - FL_DVE - FL_PE  # 10752 -> gpsimd ap_gather share
LCH = 1920         # chunk cols for the DVE label path
PCH = 1024         # chunk cols for the PE label path
GCH = 128          # cols per ap_gather instruction (out free = 16*GCH)
GLD = 1024         # cols per gather-path input DMA

_CACHE = {}

import os as _os
_NO_LABELS = bool(int(_os.environ.get("KERN_NO_LABELS", "0")))
_NO_IMG = bool(int(_os.environ.get("KERN_NO_IMG", "0")))


def _lin_weights(n_in, n_out):
    pos = np.linspace(0.0, n_in - 1.0, n_out, dtype=np.float64)
    i0 = np.clip(np.floor(pos).astype(np.int64), 0, n_in - 2)
    f = pos - i0
    Wm = np.zeros((n_out, n_in), np.float64)
    r = np.arange(n_out)
    np.add.at(Wm, (r, i0), 1.0 - f)
    np.add.at(Wm, (r, i0 + 1), f)
    return Wm


def _gauss_kernels(sigma3):
    ar = np.arange(K, dtype=np.float64) - K // 2
    out = np.zeros((3, K), np.float64)
    for i, sg in enumerate(sigma3):
        s = max(float(sg), 1e-3)
        g = np.exp(-0.5 * ar * ar / (s * s))
        g = g / g.sum()
        if float(sg) >= 0.01:
            out[i] = g
        else:
            out[i, K // 2] = 1.0
    return out


def _edge_folded_toeplitz(g, n):
    """[n, n] matrix M with out[j] = sum_i M[i, j] * x[i], replicate padding."""
    M = np.zeros((n, n), np.float64)
    for j in range(n):
        for t in range(K):
            src = min(max(j + t - P, 0), n - 1)
            M[src, j] += g[t]
    return M


def _slab_toeplitz(g):
    """[HS, HC]: slab rows (pre-clipped by host) -> interior outputs."""
    M = np.zeros((HS, HC), np.float64)
    for j in range(HC):
        for t in range(K):
            M[j + t, j] += g[t]
    return M


def _build_program():
    nc = bacc.Bacc("TRN2", target_bir_lowering=False, debug=False)

    # ---- external inputs (per core) ----
    xs_h = nc.dram_tensor("xs", [HS, D * W], BF16, kind="ExternalInput")
    c_h = nc.dram_tensor("cydw", [4, D * W], BF16, kind="ExternalInput")
    wht_h = nc.dram_tensor("wht", [4, HS], BF16, kind="ExternalInput")
    gh_h = nc.dram_tensor("gh", [HS, HC], BF16, kind="ExternalInput")
    gw0_h = nc.dram_tensor("gw0", [WIN, HC], BF16, kind="ExternalInput")
    gw1_h = nc.dram_tensor("gw1", [WIN, HC], BF16, kind="ExternalInput")
    gd0_h = nc.dram_tensor("gd0", [WIN, HC], BF16, kind="ExternalInput")
    gd1_h = nc.dram_tensor("gd1", [WIN, HC], BF16, kind="ExternalInput")
    lab_h = nc.dram_tensor("lab", [128, FLAB], I16, kind="ExternalInput")
    c16_h = nc.dram_tensor("c16", [128, 16], F32, kind="ExternalInput")
    tabf_h = nc.dram_tensor("tabf", [128, N_LABELS], F32, kind="ExternalInput")
    tab32_h = nc.dram_tensor("tab32", [128, N_LABELS], I32, kind="ExternalInput")
    idbf_h = nc.dram_tensor("idbf", [128, 128], BF16, kind="ExternalInput")

    # ---- external outputs ----
    img_h = nc.dram_tensor("img", [D, HC, W], BF16, kind="ExternalOutput")
    labo_h = nc.dram_tensor("labo", [128, FL_DVE], I16, kind="ExternalOutput")
    labp_h = nc.dram_tensor("labp", [128, FL_PE], BF16, kind="ExternalOutput")
    labg_h = nc.dram_tensor("labg", [8, 16 * FL_G], I32, kind="ExternalOutput")

    from contextlib import ExitStack
    with tile.TileContext(nc) as tc:
        with ExitStack() as stack:
            cst = stack.enter_context(tc.tile_pool(name="consts", bufs=1))
            sxp = stack.enter_context(tc.tile_pool(name="sxp", bufs=2))
            cbp = stack.enter_context(tc.tile_pool(name="cbp", bufs=2))
            ebp = stack.enter_context(tc.tile_pool(name="ebp", bufs=6))
            xbp = stack.enter_context(tc.tile_pool(name="xbp", bufs=2))
            xhp = stack.enter_context(tc.tile_pool(name="xhp", bufs=2))
            zwp = stack.enter_context(tc.tile_pool(name="zwp", bufs=2))
            y2p = stack.enter_context(tc.tile_pool(name="y2p", bufs=1))
            zdp = stack.enter_context(tc.tile_pool(name="zdp", bufs=3))
            zip_ = stack.enter_context(tc.tile_pool(name="zip", bufs=4))
            lp = stack.enter_context(tc.tile_pool(name="lp", bufs=2))
            ltmp = stack.enter_context(tc.tile_pool(name="ltmp", bufs=1))
            dkp = stack.enter_context(tc.tile_pool(name="dkp", bufs=6))
            glp = stack.enter_context(tc.tile_pool(name="glp", bufs=2))
            gop = stack.enter_context(tc.tile_pool(name="gop", bufs=3))
            psp = stack.enter_context(tc.tile_pool(name="ps", bufs=4, space="PSUM"))
            pslp = stack.enter_context(tc.tile_pool(name="psl", bufs=2, space="PSUM"))
            psdp = stack.enter_context(tc.tile_pool(name="psd", bufs=2, space="PSUM"))
            # ---- constants to SBUF ----
            ght = cst.tile([HS, HC], BF16)
            nc.sync.dma_start(ght[:], gh_h.ap())
            gw0 = cst.tile([WIN, HC], BF16)
            nc.sync.dma_start(gw0[:], gw0_h.ap())
            gw1 = cst.tile([WIN, HC], BF16)
            nc.sync.dma_start(gw1[:], gw1_h.ap())
            gd0 = cst.tile([WIN, HC], BF16)
            nc.sync.dma_start(gd0[:], gd0_h.ap())
            gd1 = cst.tile([WIN, HC], BF16)
            nc.sync.dma_start(gd1[:], gd1_h.ap())
            whtt = cst.tile([4, HS], BF16)
            nc.sync.dma_start(whtt[:], wht_h.ap())
            c16t = cst.tile([128, 16], F32)
            nc.sync.dma_start(c16t[:], c16_h.ap())
            tabf = cst.tile([128, N_LABELS], F32)
            nc.sync.dma_start(tabf[:], tabf_h.ap())
            tab32 = cst.tile([128, N_LABELS], I32)
            nc.sync.dma_start(tab32[:], tab32_h.ap())
            idbf = cst.tile([128, 128], BF16)
            nc.sync.dma_start(idbf[:], idbf_h.ap())

            # y2 SBUF-resident: w' halves on partitions 0..95, free (h', d)
            y2a = y2p.tile([HC, HC * D], BF16, tag="y2a")
            y2b = y2p.tile([HC, HC * D], BF16, tag="y2b")

            def emit_label_dve(s0, fc):
                """DVE-direct 16-entry packed LUT on labo cols [s0, s0+fc)."""
                lt = lp.tile([128, fc], I16, tag="lt")
                nc.scalar.dma_start(lt[:], lab_h.ap()[:, s0:s0 + fc])
                hh = ltmp.tile([128, fc], I16, tag="hh")
                sh = ltmp.tile([128, fc], I16, tag="sh")
                acc = ltmp.tile([128, fc], I16, tag="acc")
                ek = ltmp.tile([128, fc], I16, tag="ek")
                o16 = lp.tile([128, fc], I16, tag="o16")
                nc.vector.tensor_scalar(hh[:], lt[:], 1, None, A.logical_shift_right)
                nc.vector.tensor_scalar(sh[:], lt[:], 1, None, A.bitwise_and)
                nc.vector.tensor_scalar(sh[:], sh[:], 7, None, A.mult)
                nc.vector.tensor_scalar(acc[:], hh[:], 0, c16t[:, 0:1], A.is_equal, A.mult)
                for k in range(1, 16):
                    nc.vector.tensor_scalar(ek[:], hh[:], k, c16t[:, k:k + 1], A.is_equal, A.mult)
                    # disjoint-one-hot accumulate: paired 32-bit bitwise OR
                    # (2 int16 lanes per op; int32 adds would round via fp32)
                    nc.vector.tensor_tensor(acc[:].bitcast(I32), acc[:].bitcast(I32),
                                            ek[:].bitcast(I32), A.bitwise_or)
                nc.vector.tensor_tensor(acc[:], acc[:], sh[:], A.logical_shift_right)
                nc.vector.tensor_scalar(o16[:], acc[:], 127, None, A.bitwise_and)
                nc.sync.dma_start(labo_h.ap()[:, s0:s0 + fc], o16[:])

            def emit_label_pe(s0, fc):
                """PE path: 32 one-hot compares (DVE, table value folded in,
                <=99 so bf16-exact) -> identity-matmul PSUM accumulation.
                Compares are emitted LOOKAHEAD passes ahead of their matmuls
                so the in-order PE queue doesn't stall on DVE."""
                LOOKAHEAD = 4
                lt = lp.tile([128, fc], I16, tag="lt")
                nc.scalar.dma_start(lt[:], lab_h.ap()[:, s0:s0 + fc])
                ob = lp.tile([128, fc], BF16, tag="ob")
                nq = (fc + 511) // 512
                psls = []
                for _ in range(nq):
                    pslt = pslp.tile([128, 512], F32, tag="psl")
                    psls.append(pslt)
                dks = {}
                def emit_compare(k):
                    dk = dkp.tile([128, fc], BF16, tag="dk")
                    nc.vector.tensor_scalar(dk[:], lt[:], k, tabf[:, k:k + 1],
                                            A.is_equal, A.mult)
                    dks[k] = dk
                for k in range(LOOKAHEAD):
                    emit_compare(k)
                for k in range(N_LABELS):
                    if k + LOOKAHEAD < N_LABELS:
                        emit_compare(k + LOOKAHEAD)
                    dk = dks.pop(k)
                    for q in range(nq):
                        qn = min(512, fc - q * 512)
                        nc.tensor.matmul(psls[q][:, :qn], idbf[:],
                                         dk[:, q * 512:q * 512 + qn],
                                         start=(k == 0), stop=(k == N_LABELS - 1))
                for q in range(nq):
                    qn = min(512, fc - q * 512)
                    nc.vector.tensor_copy(ob[:, q * 512:q * 512 + qn], psls[q][:, :qn])
                nc.sync.dma_start(labp_h.ap()[:, s0 - FL_DVE:s0 - FL_DVE + fc], ob[:])

            def emit_label_gather(s0, fc):
                """GPSIMD ap_gather over gather-share cols [s0, s0+fc)."""
                ltg = glp.tile([128, fc], I16, tag="ltg")
                base = FL_DVE + FL_PE
                nc.sync.dma_start(ltg[:], lab_h.ap()[:, base + s0:base + s0 + fc])
                pend_g = None
                for g0 in list(range(0, fc, GCH)) + [None]:
                    if g0 is not None:
                        og = gop.tile([128, 16 * GCH], I32, tag="og")
                        nc.gpsimd.ap_gather(og[:], tab32[:], ltg[:, g0:g0 + GCH],
                                            channels=128, num_elems=N_LABELS, d=1,
                                            num_idxs=16 * GCH)
                    if pend_g is not None:
                        og_p, g0_p = pend_g
                        pstep = og_p[:].ap[0][0]
                        nc.sync.dma_start(
                            labg_h.ap()[:, 16 * (s0 + g0_p):16 * (s0 + g0_p + GCH)],
                            bass.AP(og_p.tensor, og_p[:].offset,
                                    [[pstep * 16, 8], [1, 16 * GCH]]),
                        )
                    pend_g = (og, g0) if g0 is not None else None

            ldve_chunks = [(s, min(LCH, FL_DVE - s)) for s in range(0, FL_DVE, LCH)]
            lpe_chunks = [(FL_DVE + s, min(PCH, FL_PE - s)) for s in range(0, FL_PE, PCH)]
            lg_chunks = [(s, min(GLD, FL_G - s)) for s in range(0, FL_G, GLD)]
            li = [0, 0, 0]

            def drip_labels(fd, fp, fg):
                # per-path completion targets (fractions of each chunk list)
                if _NO_LABELS:
                    return
                while li[0] < len(ldve_chunks) * fd:
                    emit_label_dve(*ldve_chunks[li[0]])
                    li[0] += 1
                while li[1] < len(lpe_chunks) * fp:
                    emit_label_pe(*lpe_chunks[li[1]])
                    li[1] += 1
                while li[2] < len(lg_chunks) * fg:
                    emit_label_gather(*lg_chunks[li[2]])
                    li[2] += 1

            # ================= stage A (software-pipelined) =================
            def phase_a1(ib):
                """load + bias matmul + exp; returns (sx, ebs)."""
                d0 = ib * DB
                sx = sxp.tile([HS, FA], BF16)
                nc.sync.dma_start(sx[:], xs_h.ap()[:, d0 * W:(d0 + DB) * W])
                cb = cbp.tile([4, FA], BF16)
                nc.sync.dma_start(cb[:], c_h.ap()[:, d0 * W:(d0 + DB) * W])
                ebs = []
                for q in range(FA // 512):
                    sl = slice(q * 512, (q + 1) * 512)
                    psb = psp.tile([HS, 512], F32, tag="ps")
                    nc.tensor.matmul(psb[:], whtt[:], cb[:, sl], start=True, stop=True)
                    eb = ebp.tile([HS, 512], BF16, tag="eb")
                    nc.scalar.activation(eb[:], psb[:], mybir.ActivationFunctionType.Exp)
                    ebs.append(eb)
                return sx, ebs

            def phase_a2(ib, sx, ebs):
                """mult, H-blur, T1, W-blur -> y2."""
                d0 = ib * DB
                xb = xbp.tile([HS, FA], BF16)
                for q in range(FA // 512):
                    sl = slice(q * 512, (q + 1) * 512)
                    nc.vector.tensor_tensor(xb[:, sl], sx[:, sl], ebs[q][:], A.mult)

                xh = xhp.tile([HC, FA], BF16)
                for q in range(FA // 512):
                    sl = slice(q * 512, (q + 1) * 512)
                    psh = psp.tile([HC, 512], F32, tag="ps")
                    nc.tensor.matmul(psh[:], ght[:], xb[:, sl], start=True, stop=True)
                    nc.scalar.copy(xh[:, sl], psh[:])

                # T1: w onto partitions, two 108-row windows; zw free = (dl, h')
                zw0 = zwp.tile([WIN, DB * HC], BF16, tag="zw0")
                zw1 = zwp.tile([WIN, DB * HC], BF16, tag="zw1")
                for half in range(2):
                    pt0 = psp.tile([WIN, 4 * HC], BF16, tag="ps")
                    pt1 = psp.tile([WIN, 4 * HC], BF16, tag="ps")
                    for t in range(4):
                        dl = half * 4 + t
                        nc.tensor.transpose(
                            pt0[:, t * HC:(t + 1) * HC],
                            xh[:, dl * W: dl * W + WIN], idbf[0:HC, 0:HC])
                        nc.tensor.transpose(
                            pt1[:, t * HC:(t + 1) * HC],
                            xh[:, dl * W + 84: dl * W + 192], idbf[0:HC, 0:HC])
                    nc.vector.tensor_copy(zw0[:, half * 4 * HC:(half + 1) * 4 * HC], pt0[:])
                    nc.scalar.copy(zw1[:, half * 4 * HC:(half + 1) * 4 * HC], pt1[:])

                # banded W-blur -> y2 (SBUF), out tile m covers w' in [96m, 96m+96)
                for m, (gwm, zwm, y2m) in enumerate(((gw0, zw0, y2a), (gw1, zw1, y2b))):
                    for q in range(2):
                        sl = slice(q * 4 * HC, (q + 1) * 4 * HC)
                        psw = psp.tile([HC, 4 * HC], F32, tag="ps")
                        nc.tensor.matmul(psw[:], gwm[:], zwm[:, sl], start=True, stop=True)
                        # psw free = (dl 4, h' 96) -> y2 free (h'*D + d) at d0+q*4
                        nc.scalar.copy(
                            bass.AP(y2m.tensor,
                                    y2m[:].offset + d0 + q * 4,
                                    [[y2m[:].ap[0][0], HC], [1, 4], [D, HC]]),
                            psw[:],
                        )

            pend = None
            for ib in range(NB_A + 1 if not _NO_IMG else 0):
                cur = phase_a1(ib) if ib < NB_A else None
                if pend is not None:
                    phase_a2(ib - 1, *pend)
                    u = ib / NB_A
                    drip_labels(u * 0.26, u, u * 0.55)
                pend = cur

            # ================= stage B (software-pipelined) =================
            def phase_b1(jb):
                """T2 transposes + zd copies; returns (zd0, zd1)."""
                h0 = jb * HB
                zd0 = zdp.tile([WIN, HB * W], BF16, tag="zd0")
                zd1 = zdp.tile([WIN, HB * W], BF16, tag="zd1")
                for half in range(2):
                    pt0 = psp.tile([WIN, 4 * 2 * HC], BF16, tag="ps")
                    pt1 = psp.tile([WIN, 4 * 2 * HC], BF16, tag="ps")
                    for t in range(4):
                        hl = half * 4 + t
                        hbase = (h0 + hl) * D
                        nc.tensor.transpose(
                            pt0[:, t * 2 * HC: t * 2 * HC + HC],
                            y2a[:, hbase: hbase + WIN], idbf[0:HC, 0:HC])
                        nc.tensor.transpose(
                            pt0[:, t * 2 * HC + HC: (t + 1) * 2 * HC],
                            y2b[:, hbase: hbase + WIN], idbf[0:HC, 0:HC])
                        nc.tensor.transpose(
                            pt1[:, t * 2 * HC: t * 2 * HC + HC],
                            y2a[:, hbase + 84: hbase + 192], idbf[0:HC, 0:HC])
                        nc.tensor.transpose(
                            pt1[:, t * 2 * HC + HC: (t + 1) * 2 * HC],
                            y2b[:, hbase + 84: hbase + 192], idbf[0:HC, 0:HC])
                    nc.scalar.copy(
                        zd0[:, half * 4 * W:(half + 1) * 4 * W], pt0[:])
                    nc.scalar.copy(
                        zd1[:, half * 4 * W:(half + 1) * 4 * W], pt1[:])
                return zd0, zd1

            def phase_b2(jb, zd0, zd1):
                """banded D-blur + img out; out tile m = d' in [96m, 96m+96)."""
                h0 = jb * HB
                for m, (gdm, zdm) in enumerate(((gd0, zd0), (gd1, zd1))):
                    for q in range(3):
                        sl = slice(q * 512, (q + 1) * 512)
                        psd = psdp.tile([HC, 512], F32, tag="psd")
                        nc.tensor.matmul(psd[:], gdm[:], zdm[:, sl], start=True, stop=True)
                        zi = zip_.tile([HC, 512], BF16, tag="zi")
                        if q % 2 == 0:
                            nc.scalar.copy(zi[:], psd[:])
                        else:
                            nc.vector.tensor_copy(zi[:], psd[:])
                        nc.sync.dma_start(
                            bass.AP(img_h, m * HC * HC * W + h0 * W + q * 512,
                                    [[HC * W, HC], [1, 512]]),
                            zi[:],
                        )

            pendb = None
            for jb in range(NB_B + 1 if not _NO_IMG else 0):
                curb = phase_b1(jb) if jb < NB_B else None
                if pendb is not None:
                    phase_b2(jb - 1, *pendb)
                    v = jb / NB_B
                    drip_labels(0.26 + v * 0.74, 1.0, 0.55 + v * 0.45)
                pendb = curb

            drip_labels(1.0, 1.0, 1.0)

    nc.compile()
    return nc


def _host_prep(x, small_bias, sigma01, labels, source_values, dest_values):
    Wd = _lin_weights(SMALL, D)
    Whm = _lin_weights(SMALL, H)
    Wwm = _lin_weights(SMALL, W)
    eyebf = np.eye(128, dtype=ml_dtypes.bfloat16)

    mapping = np.zeros(TABLE, np.int64)
    mapping[np.asarray(source_values, np.int64)] = np.asarray(dest_values, np.int64)
    T = mapping[:N_LABELS]
    C16 = (T[0::2] | (T[1::2] << 7)).astype(np.float32)
    c16_rep = np.broadcast_to(C16, (128, 16)).copy()
    tabf_rep = np.broadcast_to(T.astype(np.float32), (128, N_LABELS)).copy()
    tab32_rep = np.broadcast_to(T.astype(np.int32), (128, N_LABELS)).copy()

    in_maps = []
    for c in range(8):
        b, half = c // 2, c % 2
        h0 = half * HC
        hidx = np.clip(np.arange(h0 - P, h0 + HC + P), 0, H - 1)

        # x slab, h-major layout [HS, D, W] in bf16
        xs = np.ascontiguousarray(
            np.asarray(x[b, 0], np.float32)[:, hidx, :].transpose(1, 0, 2)
        ).astype(ml_dtypes.bfloat16).reshape(HS, D * W)

        sm = np.asarray(small_bias[b, 0], np.float64) * BIAS_STD
        Cydw = np.einsum("xyz,dx,wz->ydw", sm, Wd, Wwm).reshape(4, D * W)
        WhT = np.ascontiguousarray(Whm[hidx, :].T)

        g3 = _gauss_kernels(np.asarray(sigma01[b], np.float64) * MAX_SIGMA)
        Gh = _slab_toeplitz(g3[1])
        Mw = _edge_folded_toeplitz(g3[2], W)
        Md = _edge_folded_toeplitz(g3[0], D)
        Gw0 = Mw[0:WIN, 0:HC]
        Gw1 = Mw[84:192, HC:192]
        Gd0 = Md[0:WIN, 0:HC]
        Gd1 = Md[84:192, HC:192]

        lab = np.asarray(labels[b, 0][:, h0:h0 + HC, :], np.int16).reshape(128, FLAB)

        in_maps.append({
            "xs": xs,
            "cydw": Cydw.astype(ml_dtypes.bfloat16),
            "wht": WhT.astype(ml_dtypes.bfloat16),
            "gh": Gh.astype(ml_dtypes.bfloat16),
            "gw0": Gw0.astype(ml_dtypes.bfloat16),
            "gw1": Gw1.astype(ml_dtypes.bfloat16),
            "gd0": Gd0.astype(ml_dtypes.bfloat16),
            "gd1": Gd1.astype(ml_dtypes.bfloat16),
            "lab": np.ascontiguousarray(lab),
            "c16": c16_rep,
            "tabf": tabf_rep,
            "tab32": tab32_rep,
            "idbf": eyebf,
        })
    return in_maps


def kernel(x, small_bias, sigma01, labels, source_values, dest_values):
    if "nc" not in _CACHE:
        _CACHE["nc"] = _build_program()
    nc = _CACHE["nc"]

    in_maps = _host_prep(x, small_bias, sigma01, labels, source_values, dest_values)
    res = run_bass_kernel_spmd(nc, in_maps, core_ids=list(range(8)))

    img = np.empty((B, C, D, H, W), np.float32)
    labels_out = np.empty((B, C, D, H, W), np.int32)
    for c in range(8):
        b, half = c // 2, c % 2
        h0 = half * HC
        r = res.results[c]
        img[b, 0, :, h0:h0 + HC, :] = r["img"].reshape(D, HC, W).astype(np.float32)
        lo = np.empty((128, FLAB), np.int32)
        lo[:, :FL_DVE] = r["labo"].astype(np.int32)
        lo[:, FL_DVE:FL_DVE + FL_PE] = np.asarray(r["labp"], np.float32).astype(np.int32)
        # gather share: labg [8, 16*FL_G], row g holds group g's elements in
        # (slot-major, partition-interleaved) order
        lg = r["labg"].astype(np.int32).reshape(8, FL_G, 16)
        for g in range(8):
            lo[16 * g:16 * g + 16, FL_DVE + FL_PE:] = lg[g].T
        labels_out[b, 0, :, h0:h0 + HC, :] = lo.reshape(D, HC, W)
    return img, labels_out
